# revision 1
# baseline (speedup 1.0000x reference)
"""CausalWanSelfAttention Trainium2 kernel — single SPMD launch on 8 NeuronCores.

Sharding: column-parallel QKV by heads. Each core owns 2 heads: one exclusive
"F" head plus one boundary "H" head shared with a sibling core; the H head's
output-projection weight is pre-scaled by 0.5 (and its RMSNorm sum-of-squares
contribution weighted 0.5) so summing the 8 partial outputs / statistics is
exact. RMSNorm statistics are combined with one tiny cross-core AllReduce
(2x3712 floats). The block-sparse mask decomposes into 4 dense attention
groups (no masking inside a group), so softmax runs without max-subtraction
(scores are O(1) after RMSNorm; |s| <= sqrt(128)). Scores are computed in
[kv, q] layout; softmax denominators via a ones-matmul; per-query
normalization is fused into the PSUM->SBUF copy. Head dims are permuted
(even dims then odd dims) host-side so RoPE needs no strided ops. State
tokens attend only to themselves (softmax==1 -> o=v): handled on host from a
tiny exported v_state. Heavy matmuls run as float32r (full-rate fp32 mode).
"""
import sys
import numpy as np

sys.path.insert(0, "/opt/trn_rl_repo")

# ---- problem constants (hardcoded; kernel.py must be self-contained) ----
FS = 512
NIB = 3
NAPB = 32
L = 3683
LP = 3712           # 29 * 128
D = 1536
NH = 12
HD = 128
EPS = 1e-6
IB0 = FS                  # 512  image blocks start
A0 = FS + NIB * 2 * FS    # 3584 actions start
S0 = A0 + NIB * NAPB      # 3680 states start
NKT = D // 128            # 12 contraction tiles
NLT = LP // 128           # 29 L tiles
SCALE = float(1.0 / np.sqrt(HD))

CW0 = 384  # projection L-chunk width
CW2 = 256  # rope/normalize L-chunk width


def _mk_chunks(w):
    ch = [(i * w, w) for i in range(LP // w)] + [(LP - LP % w, LP % w)]
    return [(c, x) for (c, x) in ch if x > 0]

CHUNKS = _mk_chunks(CW0)
CHUNKS2 = _mk_chunks(CW2)

# core -> (F head, H head); H heads are computed on two cores each
CORE_HEADS = []
for _a in range(4):
    CORE_HEADS.append((3 * _a, 3 * _a + 1))
    CORE_HEADS.append((3 * _a + 2, 3 * _a + 1))


def _groups():
    """Dense attention groups: q ranges, kv 128-tile indices, runt kv info."""
    gs = [dict(q=[(0, 512)], kvt=list(range(4)), runt=None)]
    for b in range(NIB):
        be = IB0 + (b + 1) * 2 * FS
        kv0 = max(IB0, be - 4 * FS)
        if kv0 == IB0:
            tiles = list(range(be // 128))
        else:
            tiles = list(range(4)) + list(range(kv0 // 128, be // 128))
        q = [(IB0 + b * 2 * FS, 512), (IB0 + b * 2 * FS + 512, 512),
             (A0 + b * NAPB, NAPB)]
        gs.append(dict(q=q, kvt=tiles, runt=b))
    return gs

GROUPS = _groups()

_PROGRAM_CACHE = {}


def _build_program():
    import concourse.bacc as bacc
    import concourse.tile as tile
    from concourse import mybir

    F32 = mybir.dt.float32
    F32R = mybir.dt.float32r
    AF = mybir.ActivationFunctionType

    nc = bacc.Bacc("TRN2", target_bir_lowering=False, debug=False, num_devices=8)

    xT = nc.dram_tensor("xT", [D, LP], F32, kind="ExternalInput")
    wq = nc.dram_tensor("wq", [D, 256], F32, kind="ExternalInput")
    wk = nc.dram_tensor("wk", [D, 256], F32, kind="ExternalInput")
    wv = nc.dram_tensor("wv", [D, 256], F32, kind="ExternalInput")
    wo = nc.dram_tensor("wo", [128, 3072], F32, kind="ExternalInput")
    bqk = nc.dram_tensor("bqk", [128, 4], F32, kind="ExternalInput")
    bv128 = nc.dram_tensor("bv128", [128, 256], F32, kind="ExternalInput")
    cos_d = nc.dram_tensor("cos128", [128, LP], F32, kind="ExternalInput")
    sin_d = nc.dram_tensor("sin128", [128, LP], F32, kind="ExternalInput")
    ones2_d = nc.dram_tensor("ones2", [128, 2], F32, kind="ExternalInput")

    outp = nc.dram_tensor("outp", [D, S0], F32, kind="ExternalOutput")
    vst = nc.dram_tensor("vst", [3, 256], F32, kind="ExternalOutput")

    with tile.TileContext(nc) as tc:
        with tc.tile_pool(name="persist", bufs=1) as P, \
             tc.tile_pool(name="xin", bufs=2) as XP, \
             tc.tile_pool(name="tmp", bufs=2) as T, \
             tc.tile_pool(name="pt", bufs=3) as PT, \
             tc.tile_pool(name="osb", bufs=2) as OSB, \
             tc.tile_pool(name="ps", bufs=2, space="PSUM") as PSY, \
             tc.tile_pool(name="dram", bufs=1, space="DRAM") as DR:

            # ---------- phase-1-resident SBUF ----------
            wq_sb = P.tile([128, NKT, 256], F32R, tag="wq")
            wk_sb = P.tile([128, NKT, 256], F32R, tag="wk")
            wv_sb = P.tile([128, NKT, 256], F32R, tag="wv")
            bqk_sb = P.tile([128, 4], F32, tag="bqk")
            bv_sb = P.tile([128, 256], F32, tag="bv")
            ones2 = P.tile([128, 2], F32R, tag="ones2")
            # whole-kernel-resident
            y_q = [P.tile([128, LP], F32R, tag=f"yq{u}", name=f"yq{u}") for u in range(2)]
            y_k = [P.tile([128, LP], F32R, tag=f"yk{u}", name=f"yk{u}") for u in range(2)]
            v_sb = P.tile([128, NLT, 256], F32R, tag="vsb")

            def ldw(dst, src):
                nc.sync.dma_start(
                    dst[:],
                    src.rearrange("(kt p) c -> p kt c", p=128).bitcast(F32R))

            ldw(wq_sb, wq)
            ldw(wk_sb, wk)
            ldw(wv_sb, wv)
            nc.sync.dma_start(bqk_sb[:], bqk.ap())
            nc.sync.dma_start(bv_sb[:], bv128.ap())
            nc.sync.dma_start(ones2[:], ones2_d.ap().bitcast(F32R))

            # ---------- phase 1: projections + ssq partials ----------
            cin = DR.tile([1, 2 * LP], F32)
            cout = DR.tile([1, 2 * LP], F32)
            xTr = xT.rearrange("(kt p) l -> p kt l", p=128)
            for (c0, cw) in CHUNKS:
                xc = XP.tile([128, NKT, CW0], F32R, tag="xc")
                nc.sync.dma_start(xc[:, :, 0:cw], xTr[:, :, c0:c0 + cw].bitcast(F32R))
                for ti, (w_sb, ys) in enumerate([(wq_sb, y_q), (wk_sb, y_k)]):
                    ssq_ps = PSY.tile([1, 512], F32, tag="ssqps")
                    for u in range(2):
                        yp = PSY.tile([128, 512], F32, tag="yp")
                        for kt in range(NKT):
                            nc.tensor.matmul(
                                yp[:, 0:cw], w_sb[:, kt, u * 128:(u + 1) * 128],
                                xc[:, kt, 0:cw],
                                start=(kt == 0), stop=(kt == NKT - 1))
                        nc.vector.tensor_scalar_add(
                            ys[u][:, c0:c0 + cw], yp[:, 0:cw],
                            bqk_sb[:, 2 * ti + u:2 * ti + u + 1])
                        y2 = T.tile([128, CW0], F32R, tag="y2")
                        nc.scalar.activation(y2[:, 0:cw],
                                             ys[u][:, c0:c0 + cw].bitcast(F32),
                                             AF.Square)
                        nc.tensor.matmul(ssq_ps[:, 0:cw], ones2[:, u:u + 1],
                                         y2[:, 0:cw], start=(u == 0), stop=(u == 1),
                                         skip_group_check=True)
                    ssq_st = T.tile([1, CW0], F32, tag="ssqst")
                    nc.vector.tensor_copy(ssq_st[:, 0:cw], ssq_ps[:, 0:cw])
                    nc.sync.dma_start(cin[0:1, ti * LP + c0:ti * LP + c0 + cw], ssq_st[:, 0:cw])
                for lt in range(c0 // 128, (c0 + cw) // 128):
                    vp = PSY.tile([128, 512], F32, tag="vp", name="vp")[:, 0:256]
                    loff = lt * 128 - c0
                    for kt in range(NKT):
                        nc.tensor.matmul(vp[:], xc[:, kt, loff:loff + 128],
                                         wv_sb[:, kt, :],
                                         start=(kt == 0), stop=(kt == NKT - 1))
                    nc.vector.tensor_add(v_sb[:, lt, :], vp[:], bv_sb[:])

            nc.sync.dma_start(vst.ap(), v_sb[96:99, 28, :].bitcast(F32))

            # ---------- collective: AllReduce the ssq partials ----------
            nc.gpsimd.collective_compute(
                "AllReduce", mybir.AluOpType.add,
                replica_groups=[list(range(8))],
                ins=[cin.opt()], outs=[cout.opt()])
            eps_t = P.tile([1, 1], F32, tag="epst")
            nc.vector.memset(eps_t[:], float(EPS))

            # cos/sin (pair-duplicated across both halves) reuse weight slots
            cos_sb = P.tile([128, LP], F32, tag="wk", name="cos_sb")
            nc.sync.dma_start(cos_sb[:], cos_d.ap())
            sin_sb = P.tile([128, LP], F32, tag="wv", name="sin_sb")
            nc.sync.dma_start(sin_sb[:], sin_d.ap())

            # ---------- phase 2: normalize + rope (in place on y) ----------
            for (c0, cw) in CHUNKS2:
                for ti, ys in enumerate([y_q, y_k]):
                    s1 = T.tile([1, CW2], F32, tag="s1")
                    nc.sync.dma_start(s1[:, 0:cw],
                                      cout[0:1, ti * LP + c0:ti * LP + c0 + cw])
                    nc.scalar.activation(s1[:, 0:cw], s1[:, 0:cw], AF.Sqrt,
                                         bias=eps_t[:, 0:1], scale=float(1.0 / D))
                    nc.vector.reciprocal(s1[:, 0:cw], s1[:, 0:cw])
                    fb = T.tile([128, CW2], F32, tag="fb")
                    nc.gpsimd.partition_broadcast(fb[:, 0:cw], s1[:, 0:cw])
                    for u in range(2):
                        y = ys[u]
                        nc.vector.tensor_mul(y[:, c0:c0 + cw],
                                             y[:, c0:c0 + cw].bitcast(F32),
                                             fb[:, 0:cw])
                        ta = T.tile([128, CW2], F32, tag="ropea")
                        tb = T.tile([128, CW2], F32, tag="ropeb")
                        tbs = T.tile([128, CW2], F32, tag="ropec")
                        yv = y[:, c0:c0 + cw].bitcast(F32)
                        nc.vector.tensor_mul(ta[:, 0:cw], yv, cos_sb[:, c0:c0 + cw])
                        nc.vector.tensor_mul(tb[:, 0:cw], yv, sin_sb[:, c0:c0 + cw])
                        nc.sync.dma_start(tbs[0:64, 0:cw], tb[64:128, 0:cw])
                        nc.sync.dma_start(tbs[64:128, 0:cw], tb[0:64, 0:cw])
                        nc.vector.tensor_sub(y[0:64, c0:c0 + cw],
                                             ta[0:64, 0:cw], tbs[0:64, 0:cw])
                        nc.vector.tensor_add(y[64:128, c0:c0 + cw],
                                             ta[64:128, 0:cw], tbs[64:128, 0:cw])

            # Wo reuses the wq weight slot
            wo_sb = P.tile([128, 3072], F32R, tag="wq", name="wo_sb")
            nc.sync.dma_start(wo_sb[:], wo.ap().bitcast(F32R))

            # ---------- phase 3: attention + partial o-projection ----------
            outr = outp.rearrange("(mt p) l -> p mt l", p=128)
            for g in GROUPS:
                runts = []
                if g["runt"] is not None:
                    b = g["runt"]
                    a_lo = A0 + b * NAPB
                    s_row = S0 + b
                    for u in range(2):
                        kr = T.tile([128, 33], F32R, tag=f"kr{u}")
                        nc.vector.tensor_copy(kr[:, 0:32],
                                              y_k[u][:, a_lo:a_lo + 32].bitcast(F32))
                        nc.vector.tensor_copy(kr[:, 32:33],
                                              y_k[u][:, s_row:s_row + 1].bitcast(F32))
                        vr = T.tile([33, 256], F32R, tag=f"vr{u}")
                        # partition-shifting copies must go through DMA
                        nc.sync.dma_start(
                            vr[0:32, :], v_sb[32 * b:32 * b + 32, 28, :])
                        nc.sync.dma_start(
                            vr[32:33, :], v_sb[96 + b:97 + b, 28, :])
                        runts.append((kr, vr))

                kvts = g["kvt"] + ([None] if g["runt"] is not None else [])
                for (q0, qw) in g["q"]:
                    o_sb = []
                    for u in range(2):
                        oT_ps = PSY.tile([128, 512], F32, tag="vp", name="oT_ps")
                        sm_ps = PSY.tile([1, 512], F32, tag="ssqps", name="sm_ps")
                        for i, t in enumerate(kvts):
                            if t is None:
                                klhs = runts[u][0][:, :]
                                vlhs = runts[u][1][:, u * 128:(u + 1) * 128]
                                kvn = 33
                            else:
                                klhs = y_k[u][:, t * 128:(t + 1) * 128]
                                vlhs = v_sb[:, t, u * 128:(u + 1) * 128]
                                kvn = 128
                            s_ps = PSY.tile([128, 512], F32, tag="yp", name="s_ps")
                            nc.tensor.matmul(s_ps[0:kvn, 0:qw], klhs,
                                             y_q[u][:, q0:q0 + qw],
                                             start=True, stop=True)
                            pT = PT.tile([128, 512], F32R, tag="pT")
                            nc.scalar.activation(pT[0:kvn, 0:qw],
                                                 s_ps[0:kvn, 0:qw], AF.Exp,
                                                 scale=SCALE)
                            nc.tensor.matmul(oT_ps[:, 0:qw], vlhs, pT[0:kvn, 0:qw],
                                             start=(i == 0), stop=(i == len(kvts) - 1),
                                             skip_group_check=True)
                            nc.tensor.matmul(sm_ps[:, 0:qw], ones2[0:kvn, 0:1],
                                             pT[0:kvn, 0:qw],
                                             start=(i == 0), stop=(i == len(kvts) - 1),
                                             skip_group_check=True)
                        sm_sb = T.tile([1, 512], F32, tag="smsb")
                        nc.vector.reciprocal(sm_sb[:, 0:qw], sm_ps[:, 0:qw])
                        rb = T.tile([128, 512], F32, tag="rb")
                        nc.gpsimd.partition_broadcast(rb[:, 0:qw], sm_sb[:, 0:qw])
                        ot = OSB.tile([128, 512], F32R, tag="ot")
                        nc.vector.tensor_mul(ot[:, 0:qw], oT_ps[:, 0:qw], rb[:, 0:qw])
                        o_sb.append(ot)
                    for m in range(NKT):
                        op_ps = PSY.tile([128, 512], F32, tag="op", name="op_ps")
                        for u in range(2):
                            nc.tensor.matmul(
                                op_ps[:, 0:qw],
                                wo_sb[:, u * D + m * 128:u * D + (m + 1) * 128],
                                o_sb[u][:, 0:qw],
                                start=(u == 0), stop=(u == 1))
                        op_sb = OSB.tile([128, 512], F32, tag="opsb")
                        nc.vector.tensor_copy(op_sb[:, 0:qw], op_ps[:, 0:qw])
                        nc.sync.dma_start(outr[:, m, q0:q0 + qw], op_sb[:, 0:qw])

    nc.finalize()
    return nc


def _prep_inputs(x, freqs, freqs_action, freqs_state, Wq, bq, Wk, bk, Wv, bv,
                 Wo, bo, gq, gk):
    """Host-side input prep -> per-core in_maps. gq/gk are ones (per spec)."""
    x = np.ascontiguousarray(np.asarray(x, np.float32)[0])
    xT = np.zeros((D, LP), np.float32)
    xT[:, :L] = x.T
    f = np.concatenate([np.asarray(freqs), np.asarray(freqs_action),
                        np.asarray(freqs_state)], 0).astype(np.float32)
    f = f.reshape(L, HD // 2, 2)
    cos128 = np.zeros((128, LP), np.float32)
    sin128 = np.zeros((128, LP), np.float32)
    cos128[0:64, :L] = f[..., 0].T
    cos128[64:128, :L] = f[..., 0].T
    sin128[0:64, :L] = f[..., 1].T
    sin128[64:128, :L] = f[..., 1].T
    perm = np.concatenate([np.arange(0, HD, 2), np.arange(1, HD, 2)])
    ones2 = np.ones((128, 2), np.float32)
    ones2[:, 1] = 0.5

    Wq = np.asarray(Wq, np.float32); Wk = np.asarray(Wk, np.float32)
    Wv = np.asarray(Wv, np.float32); Wo = np.asarray(Wo, np.float32)
    bq = np.asarray(bq, np.float32); bk = np.asarray(bk, np.float32)
    bv = np.asarray(bv, np.float32)

    in_maps = []
    for c in range(8):
        F, H = CORE_HEADS[c]
        pf = F * HD + perm
        ph = H * HD + perm
        vcols = np.r_[F * HD:(F + 1) * HD, H * HD:(H + 1) * HD]
        in_maps.append({
            "xT": xT,
            "wq": np.ascontiguousarray(np.concatenate([Wq[:, pf], Wq[:, ph]], 1)),
            "wk": np.ascontiguousarray(np.concatenate([Wk[:, pf], Wk[:, ph]], 1)),
            "wv": np.ascontiguousarray(Wv[:, vcols]),
            "wo": np.ascontiguousarray(np.concatenate(
                [Wo[F * HD:(F + 1) * HD, :], 0.5 * Wo[H * HD:(H + 1) * HD, :]],
                1).astype(np.float32)),
            "bqk": np.ascontiguousarray(
                np.stack([bq[pf], bq[ph], bk[pf], bk[ph]], 1).astype(np.float32)),
            "bv128": np.ascontiguousarray(
                np.broadcast_to(bv[vcols][None, :], (128, 256))).copy(),
            "cos128": cos128, "sin128": sin128, "ones2": ones2,
        })
    return in_maps


def kernel(**inputs) -> np.ndarray:
    from concourse.bass_utils import run_bass_kernel_spmd

    if "nc" not in _PROGRAM_CACHE:
        _PROGRAM_CACHE["nc"] = _build_program()
    nc = _PROGRAM_CACHE["nc"]

    in_maps = _prep_inputs(**inputs)
    res = run_bass_kernel_spmd(nc, in_maps, core_ids=list(range(8)))

    Wo = np.asarray(inputs["Wo"], np.float32)
    bo = np.asarray(inputs["bo"], np.float32)
    out = np.zeros((L, D), np.float32)
    acc = np.zeros((D, S0), np.float32)
    for c in range(8):
        acc += res.results[c]["outp"]
    out[:S0] = acc.T
    v_state = np.zeros((3, D), np.float32)
    have = set()
    for c in range(8):
        F, H = CORE_HEADS[c]
        vs = res.results[c]["vst"]
        if F not in have:
            v_state[:, F * HD:(F + 1) * HD] = vs[:, :HD]
            have.add(F)
        if H not in have:
            v_state[:, H * HD:(H + 1) * HD] = vs[:, HD:]
            have.add(H)
    out[S0:S0 + NIB] = v_state @ Wo
    out += bo[None, :]
    return out[None].astype(np.float32)



# revision 14
# speedup vs baseline: 12.7317x; 12.7317x over previous
"""CausalWanSelfAttention Trainium2 kernel — single SPMD launch on 8 NeuronCores.

The tunneled launch is transfer-bound (~40MB/s host<->device), so the design
minimizes bytes through the tunnel:
  * all inputs ship as ONE packed fp16 tensor pair per core (x + cos/sin
    sharded 1/8th per core, per-core head-sliced weights);
  * x and cos/sin are AllGathered on device over NeuronLink;
  * partial outputs are ReduceScattered on device and returned as one fp16
    shard per core (the host only concatenates + adds bo);
  * the donated PJRT output buffers are created on device (never uploaded).

Compute sharding (as before): column-parallel QKV by heads. Each core owns 2
heads: one exclusive "F" head plus one boundary "H" head shared with a sibling
core; the H head's output-projection weight is pre-scaled by 0.5 (and its
RMSNorm sum-of-squares contribution weighted 0.5) so summing partial outputs /
statistics is exact. RMSNorm statistics are combined with one tiny cross-core
AllReduce. The block-sparse mask decomposes into 4 dense attention groups, so
softmax runs without max-subtraction. Scores are computed in [kv, q] layout;
softmax denominators via a ones-matmul; per-query normalization fused into the
PSUM->SBUF copy. Head dims are permuted (even then odd) host-side so RoPE
needs no strided ops. State tokens attend only to themselves (o=v): handled as
a tiny extra q-group on device. Heavy matmuls run fp16 (projections, o-proj)
or float32r (attention).
"""
import sys
import numpy as np

sys.path.insert(0, "/opt/trn_rl_repo")

# ---- problem constants (hardcoded; kernel.py must be self-contained) ----
FS = 512
NIB = 3
NAPB = 32
L = 3683
LP = 3712           # 29 * 128
D = 1536
NH = 12
HD = 128
EPS = 1e-6
IB0 = FS                  # 512  image blocks start
A0 = FS + NIB * 2 * FS    # 3584 actions start
S0 = A0 + NIB * NAPB      # 3680 states start
NKT = D // 128            # 12 contraction tiles
NLT = LP // 128           # 29 L tiles
SCALE = float(1.0 / np.sqrt(HD))

NC = 8
SH = LP // NC             # 464  per-core L shard width
XIN_R = D + 128           # 1664 rows: 1536 xT + 64 cos + 64 sin
WIN_R = D + 1             # 1537 rows: weights + bias row
ORS = D // NC             # 192  output rows per core after ReduceScatter

CW2 = 256  # rope/normalize L-chunk width
CHUNKS2 = ([(i * CW2, CW2) for i in range(LP // CW2)]
           + ([(LP - LP % CW2, LP % CW2)] if LP % CW2 else []))

# core -> (F head, H head); H heads are computed on two cores each
CORE_HEADS = []
for _a in range(4):
    CORE_HEADS.append((3 * _a, 3 * _a + 1))
    CORE_HEADS.append((3 * _a + 2, 3 * _a + 1))


def _groups():
    """Dense attention groups: q ranges, kv 128-tile indices, runt kv info."""
    gs = [dict(q=[(0, 512)], kvt=list(range(4)), runt=None)]
    for b in range(NIB):
        be = IB0 + (b + 1) * 2 * FS
        kv0 = max(IB0, be - 4 * FS)
        if kv0 == IB0:
            tiles = list(range(be // 128))
        else:
            tiles = list(range(4)) + list(range(kv0 // 128, be // 128))
        q = [(IB0 + b * 2 * FS, 512), (IB0 + b * 2 * FS + 512, 512),
             (A0 + b * NAPB, NAPB)]
        gs.append(dict(q=q, kvt=tiles, runt=b))
    return gs

GROUPS = _groups()

DEBUG_EXPORTS = False
_PROGRAM_CACHE = {}


def _build_program():
    import concourse.bacc as bacc
    import concourse.tile as tile
    from concourse import mybir

    F16 = mybir.dt.float16
    F32 = mybir.dt.float32
    F32R = mybir.dt.float32r
    AF = mybir.ActivationFunctionType

    nc = bacc.Bacc("TRN2", target_bir_lowering=False, debug=False, num_devices=8)

    xin = nc.dram_tensor("xin", [XIN_R, SH], F16, kind="ExternalInput")
    win = nc.dram_tensor("win", [WIN_R, 1024], F16, kind="ExternalInput")
    outp16 = nc.dram_tensor("outp16", [ORS, LP], F16, kind="ExternalOutput")
    if DEBUG_EXPORTS:
        dbgy = nc.dram_tensor("dbgy", [128, 4 * LP], F32, kind="ExternalOutput")
        dbgv = nc.dram_tensor("dbgv", [128, NLT * 256], F32, kind="ExternalOutput")

    xst = nc.dram_tensor("xst", [XIN_R, SH], F16, kind="Internal")
    xg = nc.dram_tensor("xg", [NC * XIN_R, SH], F16, kind="Internal")
    opart = nc.dram_tensor("opart", [D, LP], F32, kind="Internal")
    rs_out = nc.dram_tensor("rs_out", [ORS, LP], F32, kind="Internal")

    with tile.TileContext(nc) as tc:
        with tc.tile_pool(name="persist", bufs=1) as P, \
             tc.tile_pool(name="xin", bufs=2) as XP, \
             tc.tile_pool(name="tmp", bufs=1) as T, \
             tc.tile_pool(name="pt", bufs=3) as PT, \
             tc.tile_pool(name="osb", bufs=2) as OSB, \
             tc.tile_pool(name="ps", bufs=2, space="PSUM") as PSY, \
             tc.tile_pool(name="dram", bufs=1, space="DRAM") as DR:

            # ---------- gather x + cos/sin shards from all cores ----------
            # (collectives cannot read IO tensors: stage through Internal DRAM)
            nc.sync.dma_start(xst.ap(), xin.ap())
            nc.gpsimd.collective_compute(
                "AllGather", mybir.AluOpType.bypass,
                replica_groups=[list(range(8))],
                ins=[xst.ap()], outs=[xg.ap()])

            # ---------- weights into SBUF (fp16) ----------
            wq_sb = P.tile([128, NKT, 256], F16, tag="wq")
            wk_sb = P.tile([128, NKT, 256], F16, tag="wk")
            wv_sb = P.tile([128, NKT, 256], F16, tag="wv")
            for j, w_sb in enumerate([wq_sb, wk_sb, wv_sb]):
                nc.sync.dma_start(
                    w_sb[:],
                    win.ap()[0:D, 256 * j:256 * (j + 1)]
                       .rearrange("(kt p) c -> p kt c", p=128))
            bqk16 = T.tile([128, 4], F16, tag="b16")
            nc.sync.dma_start(
                bqk16[:],
                win.ap()[D:D + 1, 0:512].rearrange("a (p c) -> p (a c)", p=128))
            bqk_sb = P.tile([128, 4], F32, tag="bqk")
            nc.vector.tensor_copy(bqk_sb[:], bqk16[:])
            bv16 = T.tile([1, 256], F16, tag="bv16")
            nc.sync.dma_start(bv16[:], win.ap()[D:D + 1, 512:768])
            bv1 = T.tile([1, 256], F32, tag="bv1")
            nc.vector.tensor_copy(bv1[:], bv16[:])
            bv_sb = P.tile([128, 256], F32, tag="bv")
            nc.gpsimd.partition_broadcast(bv_sb[:], bv1[:])
            ones2 = P.tile([128, 2], F32, tag="ones2")
            nc.vector.memset(ones2[:, 0:1], 1.0)
            nc.vector.memset(ones2[:, 1:2], 0.5)
            eps_t = P.tile([1, 1], F32, tag="epst")
            nc.vector.memset(eps_t[:], float(EPS))

            # whole-kernel-resident
            y_q = [P.tile([128, LP], F32R, tag=f"yq{u}", name=f"yq{u}") for u in range(2)]
            y_k = [P.tile([128, LP], F32R, tag=f"yk{u}", name=f"yk{u}") for u in range(2)]
            v_sb = P.tile([128, NLT, 256], F32R, tag="vsb")

            cin = DR.tile([1, 2 * LP], F32)
            cout = DR.tile([1, 2 * LP], F32)

            # ---------- phase 1a: q/k projections + ssq partials ----------
            for s in range(NC):
                c0 = s * SH
                xc = XP.tile([128, NKT, SH], F16, tag="xc")
                nc.sync.dma_start(
                    xc[:],
                    xg.ap()[s * XIN_R:s * XIN_R + D, :]
                      .rearrange("(kt p) l -> p kt l", p=128))
                for ti, (w_sb, ys) in enumerate([(wq_sb, y_q), (wk_sb, y_k)]):
                    ssq_ps = PSY.tile([1, 512], F32, tag="ssqps")
                    for u in range(2):
                        yp = PSY.tile([128, 512], F32, tag="yp")
                        for kt in range(NKT):
                            nc.tensor.matmul(
                                yp[:, 0:SH], w_sb[:, kt, u * 128:(u + 1) * 128],
                                xc[:, kt, :],
                                start=(kt == 0), stop=(kt == NKT - 1))
                        nc.vector.tensor_scalar_add(
                            ys[u][:, c0:c0 + SH], yp[:, 0:SH],
                            bqk_sb[:, 2 * ti + u:2 * ti + u + 1])
                        y2 = T.tile([128, SH], F32R, tag="y2")
                        nc.scalar.activation(y2[:],
                                             ys[u][:, c0:c0 + SH].bitcast(F32),
                                             AF.Square)
                        nc.tensor.matmul(ssq_ps[:, 0:SH],
                                         ones2[:, u:u + 1].bitcast(F32R),
                                         y2[:], start=(u == 0), stop=(u == 1),
                                         skip_group_check=True)
                    ssq_st = T.tile([1, SH], F32, tag="ssqst")
                    nc.vector.tensor_copy(ssq_st[:], ssq_ps[:, 0:SH])
                    nc.sync.dma_start(
                        cin[0:1, ti * LP + c0:ti * LP + c0 + SH], ssq_st[:])

            # ---------- phase 1b: v per 128-wide L tile ----------
            for lt in range(NLT):
                xv = XP.tile([128, NKT, 128], F16, tag="xv")
                a = lt * 128
                while a < (lt + 1) * 128:
                    s = a // SH
                    b = min((lt + 1) * 128, (s + 1) * SH)
                    nc.sync.dma_start(
                        xv[:, :, a - lt * 128:b - lt * 128],
                        xg.ap()[s * XIN_R:s * XIN_R + D, a - s * SH:b - s * SH]
                          .rearrange("(kt p) l -> p kt l", p=128))
                    a = b
                vp = PSY.tile([128, 512], F32, tag="vp", name="vp")[:, 0:256]
                for kt in range(NKT):
                    nc.tensor.matmul(vp[:], xv[:, kt, :], wv_sb[:, kt, :],
                                     start=(kt == 0), stop=(kt == NKT - 1))
                nc.vector.tensor_add(v_sb[:, lt, :], vp[:], bv_sb[:])

            # ---------- collective: AllReduce the ssq partials ----------
            nc.gpsimd.collective_compute(
                "AllReduce", mybir.AluOpType.add,
                replica_groups=[list(range(8))],
                ins=[cin.opt()], outs=[cout.opt()])

            # ---------- cos/sin -> f32, duplicated across both halves ----------
            cos_sb = P.tile([128, LP], F32, tag="cos")
            sin_sb = P.tile([128, LP], F32, tag="sin")
            for s in range(NC):
                c0 = s * SH
                csb = T.tile([128, SH], F16, tag="csb")
                nc.sync.dma_start(
                    csb[:], xg.ap()[s * XIN_R + D:(s + 1) * XIN_R, :])
                csw = T.tile([128, SH], F16, tag="csw")
                nc.sync.dma_start(csw[64:128, :], csb[0:64, :])
                nc.sync.dma_start(csw[0:64, :], csb[64:128, :])
                nc.vector.tensor_copy(cos_sb[0:64, c0:c0 + SH], csb[0:64, :])
                nc.vector.tensor_copy(cos_sb[64:128, c0:c0 + SH], csw[64:128, :])
                nc.vector.tensor_copy(sin_sb[64:128, c0:c0 + SH], csb[64:128, :])
                nc.vector.tensor_copy(sin_sb[0:64, c0:c0 + SH], csw[0:64, :])

            # ---------- phase 2: normalize + rope (in place on y) ----------
            for (c0, cw) in CHUNKS2:
                for ti, ys in enumerate([y_q, y_k]):
                    s1 = T.tile([1, CW2], F32, tag="s1")
                    nc.sync.dma_start(s1[:, 0:cw],
                                      cout[0:1, ti * LP + c0:ti * LP + c0 + cw])
                    nc.scalar.activation(s1[:, 0:cw], s1[:, 0:cw], AF.Sqrt,
                                         bias=eps_t[:, 0:1], scale=float(1.0 / D))
                    nc.vector.reciprocal(s1[:, 0:cw], s1[:, 0:cw])
                    fb = T.tile([128, CW2], F32, tag="fb")
                    nc.gpsimd.partition_broadcast(fb[:, 0:cw], s1[:, 0:cw])
                    for u in range(2):
                        y = ys[u]
                        nc.vector.tensor_mul(y[:, c0:c0 + cw],
                                             y[:, c0:c0 + cw].bitcast(F32),
                                             fb[:, 0:cw])
                        ta = T.tile([128, CW2], F32, tag="ropea")
                        tb = T.tile([128, CW2], F32, tag="ropeb")
                        tbs = T.tile([128, CW2], F32, tag="ropec")
                        yv = y[:, c0:c0 + cw].bitcast(F32)
                        nc.vector.tensor_mul(ta[:, 0:cw], yv, cos_sb[:, c0:c0 + cw])
                        nc.vector.tensor_mul(tb[:, 0:cw], yv, sin_sb[:, c0:c0 + cw])
                        nc.sync.dma_start(tbs[0:64, 0:cw], tb[64:128, 0:cw])
                        nc.sync.dma_start(tbs[64:128, 0:cw], tb[0:64, 0:cw])
                        nc.vector.tensor_sub(y[0:64, c0:c0 + cw],
                                             ta[0:64, 0:cw], tbs[0:64, 0:cw])
                        nc.vector.tensor_add(y[64:128, c0:c0 + cw],
                                             ta[64:128, 0:cw], tbs[64:128, 0:cw])

            if DEBUG_EXPORTS:
                for u in range(2):
                    nc.sync.dma_start(dbgy.ap()[:, u * LP:(u + 1) * LP],
                                      y_q[u][:].bitcast(F32))
                    nc.sync.dma_start(
                        dbgy.ap()[:, (2 + u) * LP:(3 + u) * LP],
                        y_k[u][:].bitcast(F32))
                nc.sync.dma_start(
                    dbgv.ap().rearrange("p (t c) -> p t c", t=NLT),
                    v_sb[:].bitcast(F32))

            # Wo (fp16) reuses the wq weight slot; free dims [j, c] with
            # out-col = j*256 + c
            wo_sb = P.tile([128, NKT, 256], F16, tag="wq", name="wo_sb")
            nc.sync.dma_start(
                wo_sb[:],
                win.ap()[0:D, 768:1024].rearrange("(p j) c -> p j c", p=128))

            # ---------- phase 3: attention + partial o-projection ----------
            outr = opart.rearrange("(mt p) l -> p mt l", p=128)

            def oproj(o_sb, q0, qw):
                for m in range(NKT):
                    op_ps = PSY.tile([128, 512], F32, tag="op", name="op_ps")
                    for u in range(2):
                        nc.tensor.matmul(
                            op_ps[:, 0:qw],
                            wo_sb[:, 6 * u + m // 2,
                                  (m % 2) * 128:(m % 2) * 128 + 128],
                            o_sb[u][:, 0:qw],
                            start=(u == 0), stop=(u == 1))
                    op_sb = OSB.tile([128, 512], F32, tag="opsb")
                    nc.vector.tensor_copy(op_sb[:, 0:qw], op_ps[:, 0:qw])
                    nc.sync.dma_start(outr[:, m, q0:q0 + qw], op_sb[:, 0:qw])

            for g in GROUPS:
                runts = []
                if g["runt"] is not None:
                    b = g["runt"]
                    a_lo = A0 + b * NAPB
                    s_row = S0 + b
                    for u in range(2):
                        kr = T.tile([128, 33], F32R, tag=f"kr{u}")
                        nc.vector.tensor_copy(kr[:, 0:32],
                                              y_k[u][:, a_lo:a_lo + 32].bitcast(F32))
                        nc.vector.tensor_copy(kr[:, 32:33],
                                              y_k[u][:, s_row:s_row + 1].bitcast(F32))
                        vr = T.tile([33, 256], F32R, tag=f"vr{u}")
                        # partition-shifting copies must go through DMA
                        nc.sync.dma_start(
                            vr[0:32, :], v_sb[32 * b:32 * b + 32, 28, :])
                        nc.sync.dma_start(
                            vr[32:33, :], v_sb[96 + b:97 + b, 28, :])
                        runts.append((kr, vr))

                kvts = g["kvt"] + ([None] if g["runt"] is not None else [])
                for (q0, qw) in g["q"]:
                    o_sb = []
                    for u in range(2):
                        oT_ps = PSY.tile([128, 512], F32, tag="vp", name="oT_ps")
                        sm_ps = PSY.tile([1, 512], F32, tag="ssqps", name="sm_ps")
                        for i, t in enumerate(kvts):
                            if t is None:
                                klhs = runts[u][0][:, :]
                                vlhs = runts[u][1][:, u * 128:(u + 1) * 128]
                                kvn = 33
                            else:
                                klhs = y_k[u][:, t * 128:(t + 1) * 128]
                                vlhs = v_sb[:, t, u * 128:(u + 1) * 128]
                                kvn = 128
                            s_ps = PSY.tile([128, 512], F32, tag="yp", name="s_ps")
                            nc.tensor.matmul(s_ps[0:kvn, 0:qw], klhs,
                                             y_q[u][:, q0:q0 + qw],
                                             start=True, stop=True)
                            pT = PT.tile([128, 512], F32R, tag="pT")
                            nc.scalar.activation(pT[0:kvn, 0:qw],
                                                 s_ps[0:kvn, 0:qw], AF.Exp,
                                                 scale=SCALE)
                            nc.tensor.matmul(oT_ps[:, 0:qw], vlhs, pT[0:kvn, 0:qw],
                                             start=(i == 0), stop=(i == len(kvts) - 1),
                                             skip_group_check=True)
                            nc.tensor.matmul(sm_ps[:, 0:qw],
                                             ones2[0:kvn, 0:1].bitcast(F32R),
                                             pT[0:kvn, 0:qw],
                                             start=(i == 0), stop=(i == len(kvts) - 1),
                                             skip_group_check=True)
                        sm_sb = T.tile([1, 512], F32, tag="smsb")
                        nc.vector.reciprocal(sm_sb[:, 0:qw], sm_ps[:, 0:qw])
                        rb = T.tile([128, 512], F32, tag="rb")
                        nc.gpsimd.partition_broadcast(rb[:, 0:qw], sm_sb[:, 0:qw])
                        ot = OSB.tile([128, 512], F16, tag="ot")
                        nc.vector.tensor_mul(ot[:, 0:qw], oT_ps[:, 0:qw], rb[:, 0:qw])
                        o_sb.append(ot)
                    oproj(o_sb, q0, qw)

            # state tokens: o = v (each attends only to itself)
            o_st = []
            for u in range(2):
                vst32 = T.tile([128, 3], F32, tag=f"vst{u}")
                for i in range(NIB):
                    nc.sync.dma_start(
                        vst32[:, i:i + 1],
                        v_sb[96 + i:97 + i, 28, u * 128:(u + 1) * 128].bitcast(F32))
                ot = OSB.tile([128, 512], F16, tag="ot")
                nc.vector.tensor_copy(ot[:, 0:3], vst32[:])
                o_st.append(ot)
            oproj(o_st, S0, 3)

            # zero-fill the padding columns so RS output is deterministic
            zt = T.tile([128, 32], F32, tag="zt")
            nc.vector.memset(zt[:], 0.0)
            for m in range(NKT):
                nc.sync.dma_start(outr[:, m, L:LP], zt[:, 0:LP - L])

            # ---------- ReduceScatter partial outputs + fp16 downcast ----------
            nc.gpsimd.collective_compute(
                "ReduceScatter", mybir.AluOpType.add,
                replica_groups=[list(range(8))],
                ins=[opart.ap()], outs=[rs_out.ap()])
            for r0, rh in [(0, 128), (128, 64)]:
                for h in range(4):
                    w0 = h * 928
                    t32 = T.tile([128, 928], F32, tag="dn32")
                    nc.sync.dma_start(t32[0:rh, :],
                                      rs_out.ap()[r0:r0 + rh, w0:w0 + 928])
                    t16 = T.tile([128, 928], F16, tag="dn16")
                    nc.vector.tensor_copy(t16[0:rh, :], t32[0:rh, :])
                    nc.sync.dma_start(outp16.ap()[r0:r0 + rh, w0:w0 + 928],
                                      t16[0:rh, :])

    nc.finalize()
    return nc


def _prep_inputs(x, freqs, freqs_action, freqs_state, Wq, bq, Wk, bk, Wv, bv,
                 Wo, bo, gq, gk):
    """Host-side input packing -> concatenated global arrays for the 8 cores.

    gq/gk are ones (per spec). Returns (xin_g [8*1664, 464], win_g
    [8*1537, 1024]) fp16.
    """
    x = np.asarray(x, np.float32)[0]
    xT16 = np.zeros((D, LP), np.float16)
    xT16[:, :L] = x.T
    f = np.concatenate([np.asarray(freqs), np.asarray(freqs_action),
                        np.asarray(freqs_state)], 0).astype(np.float32)
    f = f.reshape(L, HD // 2, 2)
    cs16 = np.zeros((128, LP), np.float16)
    cs16[0:64, :L] = f[..., 0].T
    cs16[64:128, :L] = f[..., 1].T
    perm = np.concatenate([np.arange(0, HD, 2), np.arange(1, HD, 2)])

    Wq = np.asarray(Wq, np.float32); Wk = np.asarray(Wk, np.float32)
    Wv = np.asarray(Wv, np.float32); Wo = np.asarray(Wo, np.float32)
    bq = np.asarray(bq, np.float32); bk = np.asarray(bk, np.float32)
    bv = np.asarray(bv, np.float32)

    xin_g = np.empty((NC * XIN_R, SH), np.float16)
    win_g = np.zeros((NC * WIN_R, 1024), np.float16)
    for c in range(NC):
        F, H = CORE_HEADS[c]
        pf = F * HD + perm
        ph = H * HD + perm
        vcols = np.r_[F * HD:(F + 1) * HD, H * HD:(H + 1) * HD]
        xo = c * XIN_R
        xin_g[xo:xo + D] = xT16[:, c * SH:(c + 1) * SH]
        xin_g[xo + D:xo + XIN_R] = cs16[:, c * SH:(c + 1) * SH]
        wo = c * WIN_R
        win_g[wo:wo + D, 0:128] = Wq[:, pf]
        win_g[wo:wo + D, 128:256] = Wq[:, ph]
        win_g[wo:wo + D, 256:384] = Wk[:, pf]
        win_g[wo:wo + D, 384:512] = Wk[:, ph]
        win_g[wo:wo + D, 512:768] = Wv[:, vcols]
        wo_sl = np.concatenate([Wo[F * HD:(F + 1) * HD, :],
                                0.5 * Wo[H * HD:(H + 1) * HD, :]], 1)
        win_g[wo:wo + D, 768:1024] = wo_sl.reshape(D, 256)
        bqk = np.stack([bq[pf], bq[ph], bk[pf], bk[ph]], 1)
        win_g[wo + D, 0:512] = bqk.reshape(-1)
        win_g[wo + D, 512:768] = bv[vcols]
    return xin_g, win_g


def _launch(xin_g, win_g):
    """One warm device round-trip: upload packed inputs, run, download the
    reduce-scattered fp16 output [1536, 3712]."""
    import jax
    import jax.numpy as jnp
    from jax.sharding import Mesh, PartitionSpec, NamedSharding
    from jax.experimental.shard_map import shard_map
    from concourse import bass2jax
    from concourse.bass2jax import _bass_exec_p, partition_id_tensor

    from concourse import mybir

    C = _PROGRAM_CACHE
    if "sharded" not in C:
        bass2jax.install_neuronx_cc_hook()
        nc = C["nc"]
        in_names = ["xin", "win"]
        out_names = []
        out_avals = []
        for alloc in nc.m.functions[0].allocations:
            if not isinstance(alloc, mybir.MemoryLocationSet):
                continue
            if alloc.kind == "ExternalOutput":
                out_names.append(alloc.memorylocations[0].name)
                out_avals.append(jax.core.ShapedArray(
                    tuple(alloc.tensor_shape), mybir.dt.np(alloc.dtype)))
        n_outs = len(out_names)
        all_in = tuple(in_names) + tuple(out_names)
        if nc.partition_id_tensor is not None:
            all_in = all_in + (nc.partition_id_tensor.name,)

        def _body(*args):
            operands = list(args)
            if nc.partition_id_tensor is not None:
                operands.append(partition_id_tensor())
            outs = _bass_exec_p.bind(
                *operands,
                out_avals=tuple(out_avals),
                in_names=all_in,
                out_names=tuple(out_names),
                lowering_input_output_aliases=(),
                sim_require_finite=False,
                sim_require_nnan=False,
                nc=nc,
            )
            return tuple(outs)

        devices = jax.devices()[:NC]
        mesh = Mesh(np.asarray(devices), ("core",))
        pspec = PartitionSpec("core")
        C["sharded"] = jax.jit(
            shard_map(_body, mesh=mesh,
                      in_specs=(pspec,) * (2 + n_outs), out_specs=(pspec,) * n_outs,
                      check_rep=False),
            donate_argnums=tuple(range(2, 2 + n_outs)), keep_unused=True)

        def _mkzeros(avals=tuple(out_avals)):
            return tuple(jnp.zeros((NC * a.shape[0],) + a.shape[1:], a.dtype)
                         for a in avals)
        C["zeros"] = jax.jit(
            _mkzeros, out_shardings=tuple(NamedSharding(mesh, pspec)
                                          for _ in range(n_outs)))
        C["out_names"] = out_names

    z = C["zeros"]()
    outs = C["sharded"](xin_g, win_g, *z)
    res = {name: np.asarray(o) for name, o in zip(C["out_names"], outs)}
    return res


def kernel(**inputs) -> np.ndarray:
    if "nc" not in _PROGRAM_CACHE:
        _PROGRAM_CACHE["nc"] = _build_program()

    xin_g, win_g = _prep_inputs(**inputs)
    res = _launch(xin_g, win_g)

    out16 = res["outp16"]
    bo = np.asarray(inputs["bo"], np.float32)
    out = out16[:, :L].T.astype(np.float32) + bo[None, :]
    return out[None]


# revision 21
# speedup vs baseline: 13.6557x; 1.0726x over previous
"""CausalWanSelfAttention Trainium2 kernel — single SPMD launch on 8 NeuronCores.

The tunneled launch is transfer-bound (~40MB/s host<->device), so the design
minimizes bytes through the tunnel:
  * all inputs ship as ONE packed fp16 tensor pair per core (x + cos/sin
    sharded 1/8th per core, per-core head-sliced weights);
  * x and cos/sin are AllGathered on device over NeuronLink;
  * partial outputs are ReduceScattered on device and returned as one fp16
    shard per core (the host only concatenates + adds bo);
  * the donated PJRT output buffers are created on device (never uploaded).

Compute sharding (as before): column-parallel QKV by heads. Each core owns 2
heads: one exclusive "F" head plus one boundary "H" head shared with a sibling
core; the H head's output-projection weight is pre-scaled by 0.5 (and its
RMSNorm sum-of-squares contribution weighted 0.5) so summing partial outputs /
statistics is exact. RMSNorm statistics are combined with one tiny cross-core
AllReduce. The block-sparse mask decomposes into 4 dense attention groups, so
softmax runs without max-subtraction. Scores are computed in [kv, q] layout;
softmax denominators via a ones-matmul; per-query normalization fused into the
PSUM->SBUF copy. Head dims are permuted (even then odd) host-side so RoPE
needs no strided ops. State tokens attend only to themselves (o=v): handled as
a tiny extra q-group on device. Heavy matmuls run fp16 (projections, o-proj)
or float32r (attention).
"""
import sys
import numpy as np

sys.path.insert(0, "/opt/trn_rl_repo")

# ---- problem constants (hardcoded; kernel.py must be self-contained) ----
FS = 512
NIB = 3
NAPB = 32
L = 3683
LP = 3712           # 29 * 128
D = 1536
NH = 12
HD = 128
EPS = 1e-6
IB0 = FS                  # 512  image blocks start
A0 = FS + NIB * 2 * FS    # 3584 actions start
S0 = A0 + NIB * NAPB      # 3680 states start
NKT = D // 128            # 12 contraction tiles
NLT = LP // 128           # 29 L tiles
SCALE = float(1.0 / np.sqrt(HD))

NC = 8
SH = LP // NC             # 464  per-core L shard width
XIN_R = D + 128           # 1664 rows: 1536 xT + 64 cos + 64 sin
HWR = D // 2              # 768  rows of the H-weight half each core uploads
WIN_R = D + HWR + 2       # 2306 rows: F weights + H half + 2 bias rows
ORS = D // NC             # 192  output rows per core after ReduceScatter
PAIRS = [[0, 1], [2, 3], [4, 5], [6, 7]]

CW2 = 256  # rope/normalize L-chunk width
CHUNKS2 = ([(i * CW2, CW2) for i in range(LP // CW2)]
           + ([(LP - LP % CW2, LP % CW2)] if LP % CW2 else []))

# core -> (F head, H head); H heads are computed on two cores each
CORE_HEADS = []
for _a in range(4):
    CORE_HEADS.append((3 * _a, 3 * _a + 1))
    CORE_HEADS.append((3 * _a + 2, 3 * _a + 1))


def _groups():
    """Dense attention groups: q ranges, kv 128-tile indices, runt kv info."""
    gs = [dict(q=[(0, 512)], kvt=list(range(4)), runt=None)]
    for b in range(NIB):
        be = IB0 + (b + 1) * 2 * FS
        kv0 = max(IB0, be - 4 * FS)
        if kv0 == IB0:
            tiles = list(range(be // 128))
        else:
            tiles = list(range(4)) + list(range(kv0 // 128, be // 128))
        q = [(IB0 + b * 2 * FS, 512), (IB0 + b * 2 * FS + 512, 512),
             (A0 + b * NAPB, NAPB)]
        gs.append(dict(q=q, kvt=tiles, runt=b))
    return gs

GROUPS = _groups()

DEBUG_EXPORTS = False
_PROGRAM_CACHE = {}


def _build_program():
    import concourse.bacc as bacc
    import concourse.tile as tile
    from concourse import mybir

    F16 = mybir.dt.float16
    F32 = mybir.dt.float32
    F32R = mybir.dt.float32r
    AF = mybir.ActivationFunctionType

    nc = bacc.Bacc("TRN2", target_bir_lowering=False, debug=False, num_devices=8)

    xin = nc.dram_tensor("xin", [XIN_R, SH], F16, kind="ExternalInput")
    win = nc.dram_tensor("win", [WIN_R, 512], F16, kind="ExternalInput")
    outp16 = nc.dram_tensor("outp16", [ORS, L], F16, kind="ExternalOutput")
    if DEBUG_EXPORTS:
        dbgy = nc.dram_tensor("dbgy", [128, 4 * LP], F32, kind="ExternalOutput")
        dbgv = nc.dram_tensor("dbgv", [128, NLT * 256], F32, kind="ExternalOutput")

    xst = nc.dram_tensor("xst", [XIN_R, SH], F16, kind="Internal")
    xg = nc.dram_tensor("xg", [NC * XIN_R, SH], F16, kind="Internal")
    wst = nc.dram_tensor("wst", [HWR, 512], F16, kind="Internal")
    wgH = nc.dram_tensor("wgH", [D, 512], F16, kind="Internal")
    opart = nc.dram_tensor("opart", [D, LP], F32, kind="Internal")
    rs_out = nc.dram_tensor("rs_out", [ORS, LP], F32, kind="Internal")

    with tile.TileContext(nc) as tc:
        with tc.tile_pool(name="persist", bufs=1) as P, \
             tc.tile_pool(name="xin", bufs=2) as XP, \
             tc.tile_pool(name="tmp", bufs=1) as T, \
             tc.tile_pool(name="pt", bufs=3) as PT, \
             tc.tile_pool(name="osb", bufs=2) as OSB, \
             tc.tile_pool(name="ps", bufs=2, space="PSUM") as PSY, \
             tc.tile_pool(name="dram", bufs=1, space="DRAM") as DR:

            # ---------- gather x + cos/sin shards from all cores ----------
            # (collectives cannot read IO tensors: stage through Internal DRAM)
            nc.sync.dma_start(xst.ap(), xin.ap())
            nc.gpsimd.collective_compute(
                "AllGather", mybir.AluOpType.bypass,
                replica_groups=[list(range(8))],
                ins=[xst.ap()], outs=[xg.ap()])
            # pair-wise gather of the shared H-head weight half: both pair
            # cores end up with the identical full H slice (static addressing)
            nc.sync.dma_start(wst.ap(), win.ap()[D:D + HWR, :])
            nc.gpsimd.collective_compute(
                "AllGather", mybir.AluOpType.bypass,
                replica_groups=PAIRS,
                ins=[wst.ap()], outs=[wgH.ap()])

            # ---------- weights into SBUF (fp16) ----------
            wq_sb = P.tile([128, NKT, 256], F16, tag="wq")
            wk_sb = P.tile([128, NKT, 256], F16, tag="wk")
            wv_sb = P.tile([128, NKT, 256], F16, tag="wv")
            for j, w_sb in enumerate([wq_sb, wk_sb, wv_sb]):
                nc.sync.dma_start(
                    w_sb[:, :, 0:128],
                    win.ap()[0:D, 128 * j:128 * (j + 1)]
                       .rearrange("(kt p) c -> p kt c", p=128))
                nc.sync.dma_start(
                    w_sb[:, :, 128:256],
                    wgH.ap()[0:D, 128 * j:128 * (j + 1)]
                       .rearrange("(kt p) c -> p kt c", p=128))
            bqk16 = T.tile([128, 4], F16, tag="b16")
            nc.sync.dma_start(
                bqk16[:],
                win.ap()[D + HWR:D + HWR + 1, 0:512]
                   .rearrange("a (p c) -> p (a c)", p=128))
            bqk_sb = P.tile([128, 4], F32, tag="bqk")
            nc.vector.tensor_copy(bqk_sb[:], bqk16[:])
            bv16 = T.tile([1, 256], F16, tag="bv16")
            nc.sync.dma_start(bv16[:], win.ap()[D + HWR + 1:D + HWR + 2, 0:256])
            bv1 = T.tile([1, 256], F32, tag="bv1")
            nc.vector.tensor_copy(bv1[:], bv16[:])
            bv_sb = P.tile([128, 256], F32, tag="bv")
            nc.gpsimd.partition_broadcast(bv_sb[:], bv1[:])
            ones2 = P.tile([128, 2], F32, tag="ones2")
            nc.vector.memset(ones2[:, 0:1], 1.0)
            nc.vector.memset(ones2[:, 1:2], 0.5)
            eps_t = P.tile([1, 1], F32, tag="epst")
            nc.vector.memset(eps_t[:], float(EPS))

            # whole-kernel-resident
            y_q = [P.tile([128, LP], F32R, tag=f"yq{u}", name=f"yq{u}") for u in range(2)]
            y_k = [P.tile([128, LP], F32R, tag=f"yk{u}", name=f"yk{u}") for u in range(2)]
            v_sb = P.tile([128, NLT, 256], F32R, tag="vsb")

            cin = DR.tile([1, 2 * LP], F32)
            cout = DR.tile([1, 2 * LP], F32)

            # ---------- phase 1a: q/k projections + ssq partials ----------
            for s in range(NC):
                c0 = s * SH
                xc = XP.tile([128, NKT, SH], F16, tag="xc")
                nc.sync.dma_start(
                    xc[:],
                    xg.ap()[s * XIN_R:s * XIN_R + D, :]
                      .rearrange("(kt p) l -> p kt l", p=128))
                for ti, (w_sb, ys) in enumerate([(wq_sb, y_q), (wk_sb, y_k)]):
                    ssq_ps = PSY.tile([1, 512], F32, tag="ssqps")
                    for u in range(2):
                        yp = PSY.tile([128, 512], F32, tag="yp")
                        for kt in range(NKT):
                            nc.tensor.matmul(
                                yp[:, 0:SH], w_sb[:, kt, u * 128:(u + 1) * 128],
                                xc[:, kt, :],
                                start=(kt == 0), stop=(kt == NKT - 1))
                        nc.vector.tensor_scalar_add(
                            ys[u][:, c0:c0 + SH], yp[:, 0:SH],
                            bqk_sb[:, 2 * ti + u:2 * ti + u + 1])
                        y2 = T.tile([128, SH], F32R, tag="y2")
                        nc.scalar.activation(y2[:],
                                             ys[u][:, c0:c0 + SH].bitcast(F32),
                                             AF.Square)
                        nc.tensor.matmul(ssq_ps[:, 0:SH],
                                         ones2[:, u:u + 1].bitcast(F32R),
                                         y2[:], start=(u == 0), stop=(u == 1),
                                         skip_group_check=True)
                    ssq_st = T.tile([1, SH], F32, tag="ssqst")
                    nc.vector.tensor_copy(ssq_st[:], ssq_ps[:, 0:SH])
                    nc.sync.dma_start(
                        cin[0:1, ti * LP + c0:ti * LP + c0 + SH], ssq_st[:])

            # ---------- phase 1b: v per 128-wide L tile ----------
            for lt in range(NLT):
                xv = XP.tile([128, NKT, 128], F16, tag="xv")
                a = lt * 128
                while a < (lt + 1) * 128:
                    s = a // SH
                    b = min((lt + 1) * 128, (s + 1) * SH)
                    nc.sync.dma_start(
                        xv[:, :, a - lt * 128:b - lt * 128],
                        xg.ap()[s * XIN_R:s * XIN_R + D, a - s * SH:b - s * SH]
                          .rearrange("(kt p) l -> p kt l", p=128))
                    a = b
                vp = PSY.tile([128, 512], F32, tag="vp", name="vp")[:, 0:256]
                for kt in range(NKT):
                    nc.tensor.matmul(vp[:], xv[:, kt, :], wv_sb[:, kt, :],
                                     start=(kt == 0), stop=(kt == NKT - 1))
                nc.vector.tensor_add(v_sb[:, lt, :], vp[:], bv_sb[:])

            # ---------- collective: AllReduce the ssq partials ----------
            nc.gpsimd.collective_compute(
                "AllReduce", mybir.AluOpType.add,
                replica_groups=[list(range(8))],
                ins=[cin.opt()], outs=[cout.opt()])

            # ---------- cos/sin -> f32, duplicated across both halves ----------
            cos_sb = P.tile([128, LP], F32, tag="cos")
            sin_sb = P.tile([128, LP], F32, tag="sin")
            for s in range(NC):
                c0 = s * SH
                csb = T.tile([128, SH], F16, tag="csb")
                nc.sync.dma_start(
                    csb[:], xg.ap()[s * XIN_R + D:(s + 1) * XIN_R, :])
                csw = T.tile([128, SH], F16, tag="csw")
                nc.sync.dma_start(csw[64:128, :], csb[0:64, :])
                nc.sync.dma_start(csw[0:64, :], csb[64:128, :])
                nc.vector.tensor_copy(cos_sb[0:64, c0:c0 + SH], csb[0:64, :])
                nc.vector.tensor_copy(cos_sb[64:128, c0:c0 + SH], csw[64:128, :])
                nc.vector.tensor_copy(sin_sb[64:128, c0:c0 + SH], csb[64:128, :])
                nc.vector.tensor_copy(sin_sb[0:64, c0:c0 + SH], csw[0:64, :])

            # ---------- phase 2: normalize + rope (in place on y) ----------
            for (c0, cw) in CHUNKS2:
                for ti, ys in enumerate([y_q, y_k]):
                    s1 = T.tile([1, CW2], F32, tag="s1")
                    nc.sync.dma_start(s1[:, 0:cw],
                                      cout[0:1, ti * LP + c0:ti * LP + c0 + cw])
                    nc.scalar.activation(s1[:, 0:cw], s1[:, 0:cw], AF.Sqrt,
                                         bias=eps_t[:, 0:1], scale=float(1.0 / D))
                    nc.vector.reciprocal(s1[:, 0:cw], s1[:, 0:cw])
                    fb = T.tile([128, CW2], F32, tag="fb")
                    nc.gpsimd.partition_broadcast(fb[:, 0:cw], s1[:, 0:cw])
                    for u in range(2):
                        y = ys[u]
                        nc.vector.tensor_mul(y[:, c0:c0 + cw],
                                             y[:, c0:c0 + cw].bitcast(F32),
                                             fb[:, 0:cw])
                        ta = T.tile([128, CW2], F32, tag="ropea")
                        tb = T.tile([128, CW2], F32, tag="ropeb")
                        tbs = T.tile([128, CW2], F32, tag="ropec")
                        yv = y[:, c0:c0 + cw].bitcast(F32)
                        nc.vector.tensor_mul(ta[:, 0:cw], yv, cos_sb[:, c0:c0 + cw])
                        nc.vector.tensor_mul(tb[:, 0:cw], yv, sin_sb[:, c0:c0 + cw])
                        nc.sync.dma_start(tbs[0:64, 0:cw], tb[64:128, 0:cw])
                        nc.sync.dma_start(tbs[64:128, 0:cw], tb[0:64, 0:cw])
                        nc.vector.tensor_sub(y[0:64, c0:c0 + cw],
                                             ta[0:64, 0:cw], tbs[0:64, 0:cw])
                        nc.vector.tensor_add(y[64:128, c0:c0 + cw],
                                             ta[64:128, 0:cw], tbs[64:128, 0:cw])

            if DEBUG_EXPORTS:
                for u in range(2):
                    nc.sync.dma_start(dbgy.ap()[:, u * LP:(u + 1) * LP],
                                      y_q[u][:].bitcast(F32))
                    nc.sync.dma_start(
                        dbgy.ap()[:, (2 + u) * LP:(3 + u) * LP],
                        y_k[u][:].bitcast(F32))
                nc.sync.dma_start(
                    dbgv.ap().rearrange("p (t c) -> p t c", t=NLT),
                    v_sb[:].bitcast(F32))

            # Wo (fp16) reuses the wq weight slot; free dims [u, j, c] with
            # flat = u*1536 + j*128 + c  (u=0 F head, u=1 H head 0.5-scaled)
            wo_sb = P.tile([128, 2, NKT, 128], F16, tag="wq", name="wo_sb")
            nc.sync.dma_start(
                wo_sb[:, 0, :, :],
                win.ap()[0:D, 384:512].rearrange("(p j) c -> p j c", p=128))
            nc.sync.dma_start(
                wo_sb[:, 1, :, :],
                wgH.ap()[0:D, 384:512].rearrange("(p j) c -> p j c", p=128))

            # ---------- phase 3: attention + partial o-projection ----------
            outr = opart.rearrange("(mt p) l -> p mt l", p=128)

            def oproj(o_sb, q0, qw):
                for m in range(NKT):
                    op_ps = PSY.tile([128, 512], F32, tag="op", name="op_ps")
                    for u in range(2):
                        nc.tensor.matmul(
                            op_ps[:, 0:qw],
                            wo_sb[:, u, m, :],
                            o_sb[u][:, 0:qw],
                            start=(u == 0), stop=(u == 1))
                    op_sb = OSB.tile([128, 512], F32, tag="opsb")
                    nc.vector.tensor_copy(op_sb[:, 0:qw], op_ps[:, 0:qw])
                    nc.sync.dma_start(outr[:, m, q0:q0 + qw], op_sb[:, 0:qw])

            for g in GROUPS:
                runts = []
                if g["runt"] is not None:
                    b = g["runt"]
                    a_lo = A0 + b * NAPB
                    s_row = S0 + b
                    for u in range(2):
                        kr = T.tile([128, 33], F32R, tag=f"kr{u}")
                        nc.vector.tensor_copy(kr[:, 0:32],
                                              y_k[u][:, a_lo:a_lo + 32].bitcast(F32))
                        nc.vector.tensor_copy(kr[:, 32:33],
                                              y_k[u][:, s_row:s_row + 1].bitcast(F32))
                        vr = T.tile([33, 256], F32R, tag=f"vr{u}")
                        # partition-shifting copies must go through DMA
                        nc.sync.dma_start(
                            vr[0:32, :], v_sb[32 * b:32 * b + 32, 28, :])
                        nc.sync.dma_start(
                            vr[32:33, :], v_sb[96 + b:97 + b, 28, :])
                        runts.append((kr, vr))

                kvts = g["kvt"] + ([None] if g["runt"] is not None else [])
                for (q0, qw) in g["q"]:
                    o_sb = []
                    for u in range(2):
                        oT_ps = PSY.tile([128, 512], F32, tag="vp", name="oT_ps")
                        sm_ps = PSY.tile([1, 512], F32, tag="ssqps", name="sm_ps")
                        for i, t in enumerate(kvts):
                            if t is None:
                                klhs = runts[u][0][:, :]
                                vlhs = runts[u][1][:, u * 128:(u + 1) * 128]
                                kvn = 33
                            else:
                                klhs = y_k[u][:, t * 128:(t + 1) * 128]
                                vlhs = v_sb[:, t, u * 128:(u + 1) * 128]
                                kvn = 128
                            s_ps = PSY.tile([128, 512], F32, tag="yp", name="s_ps")
                            nc.tensor.matmul(s_ps[0:kvn, 0:qw], klhs,
                                             y_q[u][:, q0:q0 + qw],
                                             start=True, stop=True)
                            pT = PT.tile([128, 512], F32R, tag="pT")
                            nc.scalar.activation(pT[0:kvn, 0:qw],
                                                 s_ps[0:kvn, 0:qw], AF.Exp,
                                                 scale=SCALE)
                            nc.tensor.matmul(oT_ps[:, 0:qw], vlhs, pT[0:kvn, 0:qw],
                                             start=(i == 0), stop=(i == len(kvts) - 1),
                                             skip_group_check=True)
                            nc.tensor.matmul(sm_ps[:, 0:qw],
                                             ones2[0:kvn, 0:1].bitcast(F32R),
                                             pT[0:kvn, 0:qw],
                                             start=(i == 0), stop=(i == len(kvts) - 1),
                                             skip_group_check=True)
                        sm_sb = T.tile([1, 512], F32, tag="smsb")
                        nc.vector.reciprocal(sm_sb[:, 0:qw], sm_ps[:, 0:qw])
                        rb = T.tile([128, 512], F32, tag="rb")
                        nc.gpsimd.partition_broadcast(rb[:, 0:qw], sm_sb[:, 0:qw])
                        ot = OSB.tile([128, 512], F16, tag="ot")
                        nc.vector.tensor_mul(ot[:, 0:qw], oT_ps[:, 0:qw], rb[:, 0:qw])
                        o_sb.append(ot)
                    oproj(o_sb, q0, qw)

            # state tokens: o = v (each attends only to itself)
            o_st = []
            for u in range(2):
                vst32 = T.tile([128, 3], F32, tag=f"vst{u}")
                for i in range(NIB):
                    nc.sync.dma_start(
                        vst32[:, i:i + 1],
                        v_sb[96 + i:97 + i, 28, u * 128:(u + 1) * 128].bitcast(F32))
                ot = OSB.tile([128, 512], F16, tag="ot")
                nc.vector.tensor_copy(ot[:, 0:3], vst32[:])
                o_st.append(ot)
            oproj(o_st, S0, 3)

            # zero-fill the padding columns so RS output is deterministic
            zt = T.tile([128, 32], F32, tag="zt")
            nc.vector.memset(zt[:], 0.0)
            for m in range(NKT):
                nc.sync.dma_start(outr[:, m, L:LP], zt[:, 0:LP - L])

            # ---------- ReduceScatter partial outputs + fp16 downcast ----------
            nc.gpsimd.collective_compute(
                "ReduceScatter", mybir.AluOpType.add,
                replica_groups=[list(range(8))],
                ins=[opart.ap()], outs=[rs_out.ap()])
            for r0, rh in [(0, 128), (128, 64)]:
                for h in range(4):
                    w0 = h * 928
                    cw = min(928, L - w0)
                    t32 = T.tile([128, 928], F32, tag="dn32")
                    nc.sync.dma_start(t32[0:rh, 0:cw],
                                      rs_out.ap()[r0:r0 + rh, w0:w0 + cw])
                    t16 = T.tile([128, 928], F16, tag="dn16")
                    nc.vector.tensor_copy(t16[0:rh, 0:cw], t32[0:rh, 0:cw])
                    nc.sync.dma_start(outp16.ap()[r0:r0 + rh, w0:w0 + cw],
                                      t16[0:rh, 0:cw])

    nc.finalize()
    return nc


def _prep_inputs(x, freqs, freqs_action, freqs_state, Wq, bq, Wk, bk, Wv, bv,
                 Wo, bo, gq, gk):
    """Host-side input packing -> concatenated global arrays for the 8 cores.

    gq/gk are ones (per spec). Returns (xin_g [8*1664, 464], win_g
    [8*1537, 1024]) fp16.
    """
    x = np.asarray(x, np.float32)[0]
    xT16 = np.zeros((D, LP), np.float16)
    xT16[:, :L] = x.T
    f = np.concatenate([np.asarray(freqs), np.asarray(freqs_action),
                        np.asarray(freqs_state)], 0).astype(np.float32)
    f = f.reshape(L, HD // 2, 2)
    cs16 = np.zeros((128, LP), np.float16)
    cs16[0:64, :L] = f[..., 0].T
    cs16[64:128, :L] = f[..., 1].T
    perm = np.concatenate([np.arange(0, HD, 2), np.arange(1, HD, 2)])

    Wq = np.asarray(Wq, np.float32); Wk = np.asarray(Wk, np.float32)
    Wv = np.asarray(Wv, np.float32); Wo = np.asarray(Wo, np.float32)
    bq = np.asarray(bq, np.float32); bk = np.asarray(bk, np.float32)
    bv = np.asarray(bv, np.float32)

    xin_g = np.empty((NC * XIN_R, SH), np.float16)
    win_g = np.zeros((NC * WIN_R, 512), np.float16)
    for c in range(NC):
        F, H = CORE_HEADS[c]
        pf = F * HD + perm
        ph = H * HD + perm
        fcols = np.arange(F * HD, (F + 1) * HD)
        hcols = np.arange(H * HD, (H + 1) * HD)
        xo = c * XIN_R
        xin_g[xo:xo + D] = xT16[:, c * SH:(c + 1) * SH]
        xin_g[xo + D:xo + XIN_R] = cs16[:, c * SH:(c + 1) * SH]
        wo = c * WIN_R
        # F-head weight slices (own)
        win_g[wo:wo + D, 0:128] = Wq[:, pf]
        win_g[wo:wo + D, 128:256] = Wk[:, pf]
        win_g[wo:wo + D, 256:384] = Wv[:, fcols]
        win_g[wo:wo + D, 384:512] = Wo[F * HD:(F + 1) * HD, :].reshape(D, 128)
        # half of the pair-shared H-head slices (gathered on device)
        hpart = np.empty((D, 512), np.float32)
        hpart[:, 0:128] = Wq[:, ph]
        hpart[:, 128:256] = Wk[:, ph]
        hpart[:, 256:384] = Wv[:, hcols]
        hpart[:, 384:512] = (0.5 * Wo[H * HD:(H + 1) * HD, :]).reshape(D, 128)
        r0 = (c % 2) * HWR
        win_g[wo + D:wo + D + HWR] = hpart[r0:r0 + HWR]
        bqk = np.stack([bq[pf], bq[ph], bk[pf], bk[ph]], 1)
        win_g[wo + D + HWR, 0:512] = bqk.reshape(-1)
        win_g[wo + D + HWR + 1, 0:256] = bv[np.r_[fcols, hcols]]
    return xin_g, win_g


def _launch(xin_g, win_g):
    """One warm device round-trip: upload packed inputs, run, download the
    reduce-scattered fp16 output [1536, 3712]."""
    import jax
    import jax.numpy as jnp
    from jax.sharding import Mesh, PartitionSpec, NamedSharding
    from jax.experimental.shard_map import shard_map
    from concourse import bass2jax
    from concourse.bass2jax import _bass_exec_p, partition_id_tensor

    from concourse import mybir

    C = _PROGRAM_CACHE
    if "sharded" not in C:
        bass2jax.install_neuronx_cc_hook()
        nc = C["nc"]
        in_names = ["xin", "win"]
        out_names = []
        out_avals = []
        for alloc in nc.m.functions[0].allocations:
            if not isinstance(alloc, mybir.MemoryLocationSet):
                continue
            if alloc.kind == "ExternalOutput":
                out_names.append(alloc.memorylocations[0].name)
                out_avals.append(jax.core.ShapedArray(
                    tuple(alloc.tensor_shape), mybir.dt.np(alloc.dtype)))
        n_outs = len(out_names)
        all_in = tuple(in_names) + tuple(out_names)
        if nc.partition_id_tensor is not None:
            all_in = all_in + (nc.partition_id_tensor.name,)

        def _body(*args):
            operands = list(args)
            if nc.partition_id_tensor is not None:
                operands.append(partition_id_tensor())
            outs = _bass_exec_p.bind(
                *operands,
                out_avals=tuple(out_avals),
                in_names=all_in,
                out_names=tuple(out_names),
                lowering_input_output_aliases=(),
                sim_require_finite=False,
                sim_require_nnan=False,
                nc=nc,
            )
            return tuple(outs)

        devices = jax.devices()[:NC]
        mesh = Mesh(np.asarray(devices), ("core",))
        pspec = PartitionSpec("core")
        C["sharded"] = jax.jit(
            shard_map(_body, mesh=mesh,
                      in_specs=(pspec,) * (2 + n_outs), out_specs=(pspec,) * n_outs,
                      check_rep=False),
            donate_argnums=tuple(range(2, 2 + n_outs)), keep_unused=True)

        def _mkzeros(avals=tuple(out_avals)):
            return tuple(jnp.zeros((NC * a.shape[0],) + a.shape[1:], a.dtype)
                         for a in avals)
        C["zeros"] = jax.jit(
            _mkzeros, out_shardings=tuple(NamedSharding(mesh, pspec)
                                          for _ in range(n_outs)))
        C["out_names"] = out_names

    z = C["zeros"]()
    outs = C["sharded"](xin_g, win_g, *z)
    res = {name: np.asarray(o) for name, o in zip(C["out_names"], outs)}
    return res


def kernel(**inputs) -> np.ndarray:
    if "nc" not in _PROGRAM_CACHE:
        _PROGRAM_CACHE["nc"] = _build_program()

    xin_g, win_g = _prep_inputs(**inputs)
    res = _launch(xin_g, win_g)

    out16 = res["outp16"]
    bo = np.asarray(inputs["bo"], np.float32)
    out = out16[:, :L].T.astype(np.float32) + bo[None, :]
    return out[None]


# revision 25
# speedup vs baseline: 14.4716x; 1.0597x over previous
"""CausalWanSelfAttention Trainium2 kernel — single SPMD launch on 8 NeuronCores.

The tunneled launch is transfer-bound (~40MB/s host<->device), so the design
minimizes bytes through the tunnel:
  * all inputs ship as ONE packed fp16 tensor pair per core (x + cos/sin
    sharded 1/8th per core, per-core head-sliced weights);
  * x and cos/sin are AllGathered on device over NeuronLink;
  * partial outputs are ReduceScattered on device and returned as one fp16
    shard per core (the host only concatenates + adds bo);
  * the donated PJRT output buffers are created on device (never uploaded).

Compute sharding (as before): column-parallel QKV by heads. Each core owns 2
heads: one exclusive "F" head plus one boundary "H" head shared with a sibling
core; the H head's output-projection weight is pre-scaled by 0.5 (and its
RMSNorm sum-of-squares contribution weighted 0.5) so summing partial outputs /
statistics is exact. RMSNorm statistics are combined with one tiny cross-core
AllReduce. The block-sparse mask decomposes into 4 dense attention groups, so
softmax runs without max-subtraction. Scores are computed in [kv, q] layout;
softmax denominators via a ones-matmul; per-query normalization fused into the
PSUM->SBUF copy. Head dims are permuted (even then odd) host-side so RoPE
needs no strided ops. State tokens attend only to themselves (o=v): handled as
a tiny extra q-group on device. Heavy matmuls run fp16 (projections, o-proj)
or float32r (attention).
"""
import sys
import numpy as np

sys.path.insert(0, "/opt/trn_rl_repo")

# ---- problem constants (hardcoded; kernel.py must be self-contained) ----
FS = 512
NIB = 3
NAPB = 32
L = 3683
LP = 3712           # 29 * 128
D = 1536
NH = 12
HD = 128
EPS = 1e-6
IB0 = FS                  # 512  image blocks start
A0 = FS + NIB * 2 * FS    # 3584 actions start
S0 = A0 + NIB * NAPB      # 3680 states start
NKT = D // 128            # 12 contraction tiles
NLT = LP // 128           # 29 L tiles
SCALE = float(1.0 / np.sqrt(HD))

NC = 8
SH = LP // NC             # 464  per-core L shard width
XIN_R = D + 128           # 1664 rows: 1536 xT + 64 cos + 64 sin
HWR = D // 2              # 768  rows of the H-weight half each core uploads
WIN_R = D + HWR + 2       # 2306 rows: F weights + H half + 2 bias rows
ORS = D // NC             # 192  output rows per core after ReduceScatter
PAIRS = [[0, 1], [2, 3], [4, 5], [6, 7]]

CW2 = 256  # rope/normalize L-chunk width
CHUNKS2 = ([(i * CW2, CW2) for i in range(LP // CW2)]
           + ([(LP - LP % CW2, LP % CW2)] if LP % CW2 else []))

# core -> (F head, H head); H heads are computed on two cores each
CORE_HEADS = []
for _a in range(4):
    CORE_HEADS.append((3 * _a, 3 * _a + 1))
    CORE_HEADS.append((3 * _a + 2, 3 * _a + 1))


def _groups():
    """Dense attention groups: q ranges, kv 128-tile indices, runt kv info."""
    gs = [dict(q=[(0, 512)], kvt=list(range(4)), runt=None)]
    for b in range(NIB):
        be = IB0 + (b + 1) * 2 * FS
        kv0 = max(IB0, be - 4 * FS)
        if kv0 == IB0:
            tiles = list(range(be // 128))
        else:
            tiles = list(range(4)) + list(range(kv0 // 128, be // 128))
        q = [(IB0 + b * 2 * FS, 512), (IB0 + b * 2 * FS + 512, 512),
             (A0 + b * NAPB, NAPB)]
        gs.append(dict(q=q, kvt=tiles, runt=b))
    return gs

GROUPS = _groups()

DEBUG_EXPORTS = False
_PROGRAM_CACHE = {}


def _build_program():
    import concourse.bacc as bacc
    import concourse.tile as tile
    from concourse import mybir

    F16 = mybir.dt.float16
    F32 = mybir.dt.float32
    F32R = mybir.dt.float32r
    AF = mybir.ActivationFunctionType

    nc = bacc.Bacc("TRN2", target_bir_lowering=False, debug=False, num_devices=8)

    xin = nc.dram_tensor("xin", [XIN_R, SH], F16, kind="ExternalInput")
    win = nc.dram_tensor("win", [WIN_R, 512], F16, kind="ExternalInput")
    outp16 = nc.dram_tensor("outp16", [ORS, L], F16, kind="ExternalOutput")
    if DEBUG_EXPORTS:
        dbgy = nc.dram_tensor("dbgy", [128, 4 * LP], F32, kind="ExternalOutput")
        dbgv = nc.dram_tensor("dbgv", [128, NLT * 256], F32, kind="ExternalOutput")

    xst = nc.dram_tensor("xst", [XIN_R, SH], F16, kind="Internal")
    xg = nc.dram_tensor("xg", [NC * XIN_R, SH], F16, kind="Internal")
    wst = nc.dram_tensor("wst", [HWR, 512], F16, kind="Internal")
    wgH = nc.dram_tensor("wgH", [D, 512], F16, kind="Internal")
    opart = nc.dram_tensor("opart", [D, LP], F16, kind="Internal")
    rs_out = nc.dram_tensor("rs_out", [ORS, LP], F16, kind="Internal")

    with tile.TileContext(nc) as tc:
        with tc.tile_pool(name="persist", bufs=1) as P, \
             tc.tile_pool(name="xin", bufs=2) as XP, \
             tc.tile_pool(name="tmp", bufs=1) as T, \
             tc.tile_pool(name="pt", bufs=3) as PT, \
             tc.tile_pool(name="osb", bufs=2) as OSB, \
             tc.tile_pool(name="ps", bufs=2, space="PSUM") as PSY, \
             tc.tile_pool(name="dram", bufs=1, space="DRAM") as DR:

            # ---------- gather x + cos/sin shards from all cores ----------
            # (collectives cannot read IO tensors: stage through Internal DRAM)
            nc.sync.dma_start(xst.ap(), xin.ap())
            nc.gpsimd.collective_compute(
                "AllGather", mybir.AluOpType.bypass,
                replica_groups=[list(range(8))],
                ins=[xst.ap()], outs=[xg.ap()])
            # pair-wise gather of the shared H-head weight half: both pair
            # cores end up with the identical full H slice (static addressing)
            nc.sync.dma_start(wst.ap(), win.ap()[D:D + HWR, :])
            nc.gpsimd.collective_compute(
                "AllGather", mybir.AluOpType.bypass,
                replica_groups=PAIRS,
                ins=[wst.ap()], outs=[wgH.ap()])

            # ---------- weights into SBUF (fp16) ----------
            wq_sb = P.tile([128, NKT, 256], F16, tag="wq")
            wk_sb = P.tile([128, NKT, 256], F16, tag="wk")
            wv_sb = P.tile([128, NKT, 256], F16, tag="wv")
            for j, w_sb in enumerate([wq_sb, wk_sb, wv_sb]):
                nc.sync.dma_start(
                    w_sb[:, :, 0:128],
                    win.ap()[0:D, 128 * j:128 * (j + 1)]
                       .rearrange("(kt p) c -> p kt c", p=128))
                nc.sync.dma_start(
                    w_sb[:, :, 128:256],
                    wgH.ap()[0:D, 128 * j:128 * (j + 1)]
                       .rearrange("(kt p) c -> p kt c", p=128))
            bqk16 = T.tile([128, 4], F16, tag="b16")
            nc.sync.dma_start(
                bqk16[:],
                win.ap()[D + HWR:D + HWR + 1, 0:512]
                   .rearrange("a (p c) -> p (a c)", p=128))
            bqk_sb = P.tile([128, 4], F32, tag="bqk")
            nc.vector.tensor_copy(bqk_sb[:], bqk16[:])
            bv16 = T.tile([1, 256], F16, tag="bv16")
            nc.sync.dma_start(bv16[:], win.ap()[D + HWR + 1:D + HWR + 2, 0:256])
            bv1 = T.tile([1, 256], F32, tag="bv1")
            nc.vector.tensor_copy(bv1[:], bv16[:])
            bv_sb = P.tile([128, 256], F32, tag="bv")
            nc.gpsimd.partition_broadcast(bv_sb[:], bv1[:])
            ones2 = P.tile([128, 2], F32, tag="ones2")
            nc.vector.memset(ones2[:, 0:1], 1.0)
            nc.vector.memset(ones2[:, 1:2], 0.5)
            eps_t = P.tile([1, 1], F32, tag="epst")
            nc.vector.memset(eps_t[:], float(EPS))

            # whole-kernel-resident
            y_q = [P.tile([128, LP], F32R, tag=f"yq{u}", name=f"yq{u}") for u in range(2)]
            y_k = [P.tile([128, LP], F32R, tag=f"yk{u}", name=f"yk{u}") for u in range(2)]
            v_sb = P.tile([128, NLT, 256], F32R, tag="vsb")

            cin = DR.tile([1, 2 * LP], F32)
            cout = DR.tile([1, 2 * LP], F32)

            # ---------- phase 1a: q/k projections + ssq partials ----------
            for s in range(NC):
                c0 = s * SH
                xc = XP.tile([128, NKT, SH], F16, tag="xc")
                nc.sync.dma_start(
                    xc[:],
                    xg.ap()[s * XIN_R:s * XIN_R + D, :]
                      .rearrange("(kt p) l -> p kt l", p=128))
                for ti, (w_sb, ys) in enumerate([(wq_sb, y_q), (wk_sb, y_k)]):
                    ssq_ps = PSY.tile([1, 512], F32, tag="ssqps")
                    for u in range(2):
                        yp = PSY.tile([128, 512], F32, tag="yp")
                        for kt in range(NKT):
                            nc.tensor.matmul(
                                yp[:, 0:SH], w_sb[:, kt, u * 128:(u + 1) * 128],
                                xc[:, kt, :],
                                start=(kt == 0), stop=(kt == NKT - 1))
                        nc.vector.tensor_scalar_add(
                            ys[u][:, c0:c0 + SH], yp[:, 0:SH],
                            bqk_sb[:, 2 * ti + u:2 * ti + u + 1])
                        y2 = T.tile([128, SH], F32R, tag="y2")
                        nc.scalar.activation(y2[:],
                                             ys[u][:, c0:c0 + SH].bitcast(F32),
                                             AF.Square)
                        nc.tensor.matmul(ssq_ps[:, 0:SH],
                                         ones2[:, u:u + 1].bitcast(F32R),
                                         y2[:], start=(u == 0), stop=(u == 1),
                                         skip_group_check=True)
                    ssq_st = T.tile([1, SH], F32, tag="ssqst")
                    nc.vector.tensor_copy(ssq_st[:], ssq_ps[:, 0:SH])
                    nc.sync.dma_start(
                        cin[0:1, ti * LP + c0:ti * LP + c0 + SH], ssq_st[:])

            # ---------- phase 1b: v per 128-wide L tile ----------
            for lt in range(NLT):
                xv = XP.tile([128, NKT, 128], F16, tag="xv")
                a = lt * 128
                while a < (lt + 1) * 128:
                    s = a // SH
                    b = min((lt + 1) * 128, (s + 1) * SH)
                    nc.sync.dma_start(
                        xv[:, :, a - lt * 128:b - lt * 128],
                        xg.ap()[s * XIN_R:s * XIN_R + D, a - s * SH:b - s * SH]
                          .rearrange("(kt p) l -> p kt l", p=128))
                    a = b
                vp = PSY.tile([128, 512], F32, tag="vp", name="vp")[:, 0:256]
                for kt in range(NKT):
                    nc.tensor.matmul(vp[:], xv[:, kt, :], wv_sb[:, kt, :],
                                     start=(kt == 0), stop=(kt == NKT - 1))
                nc.vector.tensor_add(v_sb[:, lt, :], vp[:], bv_sb[:])

            # ---------- collective: AllReduce the ssq partials ----------
            nc.gpsimd.collective_compute(
                "AllReduce", mybir.AluOpType.add,
                replica_groups=[list(range(8))],
                ins=[cin.opt()], outs=[cout.opt()])

            # ---------- cos/sin -> f32, duplicated across both halves ----------
            cos_sb = P.tile([128, LP], F32, tag="cos")
            sin_sb = P.tile([128, LP], F32, tag="sin")
            for s in range(NC):
                c0 = s * SH
                csb = T.tile([128, SH], F16, tag="csb")
                nc.sync.dma_start(
                    csb[:], xg.ap()[s * XIN_R + D:(s + 1) * XIN_R, :])
                csw = T.tile([128, SH], F16, tag="csw")
                nc.sync.dma_start(csw[64:128, :], csb[0:64, :])
                nc.sync.dma_start(csw[0:64, :], csb[64:128, :])
                nc.vector.tensor_copy(cos_sb[0:64, c0:c0 + SH], csb[0:64, :])
                nc.vector.tensor_copy(cos_sb[64:128, c0:c0 + SH], csw[64:128, :])
                nc.vector.tensor_copy(sin_sb[64:128, c0:c0 + SH], csb[64:128, :])
                nc.vector.tensor_copy(sin_sb[0:64, c0:c0 + SH], csw[0:64, :])

            # ---------- phase 2: normalize + rope (in place on y) ----------
            for (c0, cw) in CHUNKS2:
                for ti, ys in enumerate([y_q, y_k]):
                    s1 = T.tile([1, CW2], F32, tag="s1")
                    nc.sync.dma_start(s1[:, 0:cw],
                                      cout[0:1, ti * LP + c0:ti * LP + c0 + cw])
                    nc.scalar.activation(s1[:, 0:cw], s1[:, 0:cw], AF.Sqrt,
                                         bias=eps_t[:, 0:1], scale=float(1.0 / D))
                    nc.vector.reciprocal(s1[:, 0:cw], s1[:, 0:cw])
                    fb = T.tile([128, CW2], F32, tag="fb")
                    nc.gpsimd.partition_broadcast(fb[:, 0:cw], s1[:, 0:cw])
                    for u in range(2):
                        y = ys[u]
                        nc.vector.tensor_mul(y[:, c0:c0 + cw],
                                             y[:, c0:c0 + cw].bitcast(F32),
                                             fb[:, 0:cw])
                        ta = T.tile([128, CW2], F32, tag="ropea")
                        tb = T.tile([128, CW2], F32, tag="ropeb")
                        tbs = T.tile([128, CW2], F32, tag="ropec")
                        yv = y[:, c0:c0 + cw].bitcast(F32)
                        nc.vector.tensor_mul(ta[:, 0:cw], yv, cos_sb[:, c0:c0 + cw])
                        nc.vector.tensor_mul(tb[:, 0:cw], yv, sin_sb[:, c0:c0 + cw])
                        nc.sync.dma_start(tbs[0:64, 0:cw], tb[64:128, 0:cw])
                        nc.sync.dma_start(tbs[64:128, 0:cw], tb[0:64, 0:cw])
                        nc.vector.tensor_sub(y[0:64, c0:c0 + cw],
                                             ta[0:64, 0:cw], tbs[0:64, 0:cw])
                        nc.vector.tensor_add(y[64:128, c0:c0 + cw],
                                             ta[64:128, 0:cw], tbs[64:128, 0:cw])

            if DEBUG_EXPORTS:
                for u in range(2):
                    nc.sync.dma_start(dbgy.ap()[:, u * LP:(u + 1) * LP],
                                      y_q[u][:].bitcast(F32))
                    nc.sync.dma_start(
                        dbgy.ap()[:, (2 + u) * LP:(3 + u) * LP],
                        y_k[u][:].bitcast(F32))
                nc.sync.dma_start(
                    dbgv.ap().rearrange("p (t c) -> p t c", t=NLT),
                    v_sb[:].bitcast(F32))

            # Wo (fp16) reuses the wq weight slot; free dims [u, j, c] with
            # flat = u*1536 + j*128 + c  (u=0 F head, u=1 H head 0.5-scaled)
            wo_sb = P.tile([128, 2, NKT, 128], F16, tag="wq", name="wo_sb")
            nc.sync.dma_start(
                wo_sb[:, 0, :, :],
                win.ap()[0:D, 384:512].rearrange("(p j) c -> p j c", p=128))
            nc.sync.dma_start(
                wo_sb[:, 1, :, :],
                wgH.ap()[0:D, 384:512].rearrange("(p j) c -> p j c", p=128))

            # ---------- phase 3: attention + partial o-projection ----------
            outr = opart.rearrange("(mt p) l -> p mt l", p=128)

            def oproj(o_sb, q0, qw):
                for m in range(NKT):
                    op_ps = PSY.tile([128, 512], F32, tag="op", name="op_ps")
                    for u in range(2):
                        nc.tensor.matmul(
                            op_ps[:, 0:qw],
                            wo_sb[:, u, m, :],
                            o_sb[u][:, 0:qw],
                            start=(u == 0), stop=(u == 1))
                    op_sb = OSB.tile([128, 512], F16, tag="opsb")
                    nc.vector.tensor_copy(op_sb[:, 0:qw], op_ps[:, 0:qw])
                    nc.sync.dma_start(outr[:, m, q0:q0 + qw], op_sb[:, 0:qw])

            for g in GROUPS:
                runts = []
                if g["runt"] is not None:
                    b = g["runt"]
                    a_lo = A0 + b * NAPB
                    s_row = S0 + b
                    for u in range(2):
                        kr = T.tile([128, 33], F32R, tag=f"kr{u}")
                        nc.vector.tensor_copy(kr[:, 0:32],
                                              y_k[u][:, a_lo:a_lo + 32].bitcast(F32))
                        nc.vector.tensor_copy(kr[:, 32:33],
                                              y_k[u][:, s_row:s_row + 1].bitcast(F32))
                        vr = T.tile([33, 256], F32R, tag=f"vr{u}")
                        # partition-shifting copies must go through DMA
                        nc.sync.dma_start(
                            vr[0:32, :], v_sb[32 * b:32 * b + 32, 28, :])
                        nc.sync.dma_start(
                            vr[32:33, :], v_sb[96 + b:97 + b, 28, :])
                        runts.append((kr, vr))

                kvts = g["kvt"] + ([None] if g["runt"] is not None else [])
                for (q0, qw) in g["q"]:
                    o_sb = []
                    for u in range(2):
                        oT_ps = PSY.tile([128, 512], F32, tag="vp", name="oT_ps")
                        sm_ps = PSY.tile([1, 512], F32, tag="ssqps", name="sm_ps")
                        for i, t in enumerate(kvts):
                            if t is None:
                                klhs = runts[u][0][:, :]
                                vlhs = runts[u][1][:, u * 128:(u + 1) * 128]
                                kvn = 33
                            else:
                                klhs = y_k[u][:, t * 128:(t + 1) * 128]
                                vlhs = v_sb[:, t, u * 128:(u + 1) * 128]
                                kvn = 128
                            s_ps = PSY.tile([128, 512], F32, tag="yp", name="s_ps")
                            nc.tensor.matmul(s_ps[0:kvn, 0:qw], klhs,
                                             y_q[u][:, q0:q0 + qw],
                                             start=True, stop=True)
                            pT = PT.tile([128, 512], F32R, tag="pT")
                            nc.scalar.activation(pT[0:kvn, 0:qw],
                                                 s_ps[0:kvn, 0:qw], AF.Exp,
                                                 scale=SCALE)
                            nc.tensor.matmul(oT_ps[:, 0:qw], vlhs, pT[0:kvn, 0:qw],
                                             start=(i == 0), stop=(i == len(kvts) - 1),
                                             skip_group_check=True)
                            nc.tensor.matmul(sm_ps[:, 0:qw],
                                             ones2[0:kvn, 0:1].bitcast(F32R),
                                             pT[0:kvn, 0:qw],
                                             start=(i == 0), stop=(i == len(kvts) - 1),
                                             skip_group_check=True)
                        sm_sb = T.tile([1, 512], F32, tag="smsb")
                        nc.vector.reciprocal(sm_sb[:, 0:qw], sm_ps[:, 0:qw])
                        rb = T.tile([128, 512], F32, tag="rb")
                        nc.gpsimd.partition_broadcast(rb[:, 0:qw], sm_sb[:, 0:qw])
                        ot = OSB.tile([128, 512], F16, tag="ot")
                        nc.vector.tensor_mul(ot[:, 0:qw], oT_ps[:, 0:qw], rb[:, 0:qw])
                        o_sb.append(ot)
                    oproj(o_sb, q0, qw)

            # state tokens: o = v (each attends only to itself)
            o_st = []
            for u in range(2):
                vst32 = T.tile([128, 3], F32, tag=f"vst{u}")
                for i in range(NIB):
                    nc.sync.dma_start(
                        vst32[:, i:i + 1],
                        v_sb[96 + i:97 + i, 28, u * 128:(u + 1) * 128].bitcast(F32))
                ot = OSB.tile([128, 512], F16, tag="ot")
                nc.vector.tensor_copy(ot[:, 0:3], vst32[:])
                o_st.append(ot)
            oproj(o_st, S0, 3)

            # zero-fill the padding columns so RS output is deterministic
            zt = T.tile([128, 32], F16, tag="zt")
            nc.vector.memset(zt[:], 0.0)
            for m in range(NKT):
                nc.sync.dma_start(outr[:, m, L:LP], zt[:, 0:LP - L])

            # ---------- ReduceScatter partial outputs (fp16) ----------
            nc.gpsimd.collective_compute(
                "ReduceScatter", mybir.AluOpType.add,
                replica_groups=[list(range(8))],
                ins=[opart.ap()], outs=[rs_out.ap()])
            nc.sync.dma_start(outp16.ap(), rs_out.ap()[:, 0:L])

    nc.finalize()
    return nc


def _prep_inputs(x, freqs, freqs_action, freqs_state, Wq, bq, Wk, bk, Wv, bv,
                 Wo, bo, gq, gk):
    """Host-side input packing -> concatenated global arrays for the 8 cores.

    gq/gk are ones (per spec). Returns (xin_g [8*1664, 464], win_g
    [8*1537, 1024]) fp16.
    """
    x = np.asarray(x, np.float32)[0]
    xT16 = np.zeros((D, LP), np.float16)
    xT16[:, :L] = x.T
    f = np.concatenate([np.asarray(freqs), np.asarray(freqs_action),
                        np.asarray(freqs_state)], 0).astype(np.float32)
    f = f.reshape(L, HD // 2, 2)
    cs16 = np.zeros((128, LP), np.float16)
    cs16[0:64, :L] = f[..., 0].T
    cs16[64:128, :L] = f[..., 1].T
    perm = np.concatenate([np.arange(0, HD, 2), np.arange(1, HD, 2)])

    Wq = np.asarray(Wq, np.float32); Wk = np.asarray(Wk, np.float32)
    Wv = np.asarray(Wv, np.float32); Wo = np.asarray(Wo, np.float32)
    bq = np.asarray(bq, np.float32); bk = np.asarray(bk, np.float32)
    bv = np.asarray(bv, np.float32)

    xin_g = np.empty((NC * XIN_R, SH), np.float16)
    win_g = np.zeros((NC * WIN_R, 512), np.float16)
    for c in range(NC):
        F, H = CORE_HEADS[c]
        pf = F * HD + perm
        ph = H * HD + perm
        fcols = np.arange(F * HD, (F + 1) * HD)
        hcols = np.arange(H * HD, (H + 1) * HD)
        xo = c * XIN_R
        xin_g[xo:xo + D] = xT16[:, c * SH:(c + 1) * SH]
        xin_g[xo + D:xo + XIN_R] = cs16[:, c * SH:(c + 1) * SH]
        wo = c * WIN_R
        # F-head weight slices (own)
        win_g[wo:wo + D, 0:128] = Wq[:, pf]
        win_g[wo:wo + D, 128:256] = Wk[:, pf]
        win_g[wo:wo + D, 256:384] = Wv[:, fcols]
        win_g[wo:wo + D, 384:512] = Wo[F * HD:(F + 1) * HD, :].reshape(D, 128)
        # half of the pair-shared H-head slices (gathered on device)
        hpart = np.empty((D, 512), np.float32)
        hpart[:, 0:128] = Wq[:, ph]
        hpart[:, 128:256] = Wk[:, ph]
        hpart[:, 256:384] = Wv[:, hcols]
        hpart[:, 384:512] = (0.5 * Wo[H * HD:(H + 1) * HD, :]).reshape(D, 128)
        r0 = (c % 2) * HWR
        win_g[wo + D:wo + D + HWR] = hpart[r0:r0 + HWR]
        bqk = np.stack([bq[pf], bq[ph], bk[pf], bk[ph]], 1)
        win_g[wo + D + HWR, 0:512] = bqk.reshape(-1)
        win_g[wo + D + HWR + 1, 0:256] = bv[np.r_[fcols, hcols]]
    return xin_g, win_g


def _launch(xin_g, win_g):
    """One warm device round-trip: upload packed inputs, run, download the
    reduce-scattered fp16 output [1536, 3712]."""
    import jax
    import jax.numpy as jnp
    from jax.sharding import Mesh, PartitionSpec, NamedSharding
    from jax.experimental.shard_map import shard_map
    from concourse import bass2jax
    from concourse.bass2jax import _bass_exec_p, partition_id_tensor

    from concourse import mybir

    C = _PROGRAM_CACHE
    if "sharded" not in C:
        bass2jax.install_neuronx_cc_hook()
        nc = C["nc"]
        in_names = ["xin", "win"]
        out_names = []
        out_avals = []
        for alloc in nc.m.functions[0].allocations:
            if not isinstance(alloc, mybir.MemoryLocationSet):
                continue
            if alloc.kind == "ExternalOutput":
                out_names.append(alloc.memorylocations[0].name)
                out_avals.append(jax.core.ShapedArray(
                    tuple(alloc.tensor_shape), mybir.dt.np(alloc.dtype)))
        n_outs = len(out_names)
        all_in = tuple(in_names) + tuple(out_names)
        if nc.partition_id_tensor is not None:
            all_in = all_in + (nc.partition_id_tensor.name,)

        def _body(*args):
            operands = list(args)
            if nc.partition_id_tensor is not None:
                operands.append(partition_id_tensor())
            outs = _bass_exec_p.bind(
                *operands,
                out_avals=tuple(out_avals),
                in_names=all_in,
                out_names=tuple(out_names),
                lowering_input_output_aliases=(),
                sim_require_finite=False,
                sim_require_nnan=False,
                nc=nc,
            )
            return tuple(outs)

        devices = jax.devices()[:NC]
        mesh = Mesh(np.asarray(devices), ("core",))
        pspec = PartitionSpec("core")
        C["sharded"] = jax.jit(
            shard_map(_body, mesh=mesh,
                      in_specs=(pspec,) * (2 + n_outs), out_specs=(pspec,) * n_outs,
                      check_rep=False),
            donate_argnums=tuple(range(2, 2 + n_outs)), keep_unused=True)

        def _mkzeros(avals=tuple(out_avals)):
            return tuple(jnp.zeros((NC * a.shape[0],) + a.shape[1:], a.dtype)
                         for a in avals)
        C["zeros"] = jax.jit(
            _mkzeros, out_shardings=tuple(NamedSharding(mesh, pspec)
                                          for _ in range(n_outs)))
        C["out_names"] = out_names

    # donated output buffers are device-created; prefetch the next call's set
    # so their (tiny) creation overlaps this call's execution
    z = C.pop("pending_z", None) or C["zeros"]()
    outs = C["sharded"](xin_g, win_g, *z)
    C["pending_z"] = C["zeros"]()
    res = {name: np.asarray(o) for name, o in zip(C["out_names"], outs)}
    return res


def kernel(**inputs) -> np.ndarray:
    if "nc" not in _PROGRAM_CACHE:
        _PROGRAM_CACHE["nc"] = _build_program()

    xin_g, win_g = _prep_inputs(**inputs)
    res = _launch(xin_g, win_g)

    out16 = res["outp16"]
    bo = np.asarray(inputs["bo"], np.float32)
    out = out16[:, :L].T.astype(np.float32) + bo[None, :]
    return out[None]
